# revision 1
# baseline (speedup 1.0000x reference)
"""Trainium2 Bass kernel for NeuralSymbolicMP layer (gnn_message_passing).

Batch-sharded over B across 8 NeuronCores. Each core processes 32 (atomic,
batch) rows against all N entities.

Layout convention on-chip: entity axis n = t*128 + p with p the SBUF
partition and t in [0, 391); fuzzy tensors live as [128, 391, 32] f32.
"""

import numpy as np
import ml_dtypes

A, B, N, D, E = 2, 128, 50000, 512, 4096
CLIP = 1e-14

NCORES = 8
BLOC = B // NCORES          # 16 batch per core
R = A * BLOC                # 32 rows per core
P = 128
T = 391                     # n tiles: 391*128 = 50048
NPAD = T * P                # 50048
NWIN = 13                   # t-windows of 32 (13*32 = 416 >= 391)
WCHUNK = 4                  # edge chunks of 128 per window (max 512 edges/win)
ECHUNKS = NWIN * WCHUNK     # 52 chunks per row
EPAD = ECHUNKS * P          # 6656 padded edge slots per row
GSPLIT = NPAD // 16         # 3128 per-partition split for gather data
TPAD = NWIN * 32            # 416

bf16 = ml_dtypes.bfloat16


def _prep_core(core, entity_pad, head_vector, head_emb, pred_emb,
               edge_val, edge_src, edge_dst):
    """Build per-core host arrays. Rows r = a*BLOC + i -> (a, core*BLOC + i)."""
    b0 = core * BLOC
    # [R, ...] row-major slices
    hv = np.zeros((R, NPAD), np.float32)
    ev = np.empty((R, E), np.float32)
    es = np.empty((R, E), np.int64)
    ed = np.empty((R, E), np.int64)
    for a in range(A):
        hv[a * BLOC:(a + 1) * BLOC, :N] = head_vector[a, b0:b0 + BLOC]
        ev[a * BLOC:(a + 1) * BLOC] = edge_val[a, b0:b0 + BLOC]
        es[a * BLOC:(a + 1) * BLOC] = edge_src[a, b0:b0 + BLOC]
        ed[a * BLOC:(a + 1) * BLOC] = edge_dst[a, b0:b0 + BLOC]

    hemb = np.empty((R, D), np.float32)
    pemb = np.empty((R, D), np.float32)
    for a in range(A):
        hemb[a * BLOC:(a + 1) * BLOC] = head_emb[a, b0:b0 + BLOC]
        pemb[a * BLOC:(a + 1) * BLOC] = pred_emb[a, b0:b0 + BLOC]

    # --- edge slotting: window w = (dst//128)//32, 4 chunks of 128 per window
    eval_pad = np.zeros((R, EPAD), np.float32)
    src_pad = np.zeros((R, EPAD), np.int64)
    used = np.zeros((R, EPAD), bool)
    dstp_pad = np.zeros((R, EPAD), np.int64)
    dstt_pad = np.zeros((R, EPAD), np.int64)
    for r in range(R):
        dt_ = ed[r] // P          # t in [0, 391)
        dp_ = ed[r] % P
        w_ = dt_ // 32            # window in [0, 13)
        order = np.argsort(w_, kind="stable")
        cnt = np.bincount(w_, minlength=NWIN)
        assert cnt.max() <= WCHUNK * P, f"window overflow {cnt.max()}"
        pos = 0
        for w in range(NWIN):
            sel = order[pos:pos + cnt[w]]
            pos += cnt[w]
            base = w * WCHUNK * P
            idx = base + np.arange(cnt[w])
            eval_pad[r, idx] = ev[r, sel]
            src_pad[r, idx] = es[r, sel]
            dstp_pad[r, idx] = dp_[sel]
            dstt_pad[r, idx] = dt_[sel] - 32 * w
            used[r, idx] = True

    # one-hot stationary S and moving W, laid out partition-contiguous:
    # Sh[r, e, c*128 + p], Wh[r, e, c*32 + t]
    S = np.zeros((R, P, ECHUNKS, P), bf16)
    W = np.zeros((R, P, ECHUNKS, 32), bf16)
    rr, jj = np.nonzero(used)
    cc = jj // P
    ee = jj % P
    S[rr, ee, cc, dstp_pad[rr, jj]] = 1
    W[rr, ee, cc, dstt_pad[rr, jj]] = 1
    S = S.reshape(R, P, ECHUNKS * P)
    W = W.reshape(R, P, ECHUNKS * 32)

    # --- gather arrays: 4 passes x 8 rows; group g holds row 8q+g split 16 ways
    hvg = np.zeros((4, P, GSPLIT), np.float32)
    gidx = np.zeros((4, P, EPAD // 16), np.uint16)
    gmask = np.zeros((4, P, EPAD), np.float32)
    for q in range(4):
        for g in range(8):
            r = 8 * q + g
            hvg[q, 16 * g:16 * (g + 1)] = hv[r].reshape(16, GSPLIT)
            idx = (src_pad[r] % GSPLIT).astype(np.uint16)
            # wrapped: index j lives at partition 16g + j%16, slot j//16
            gidx[q, 16 * g:16 * (g + 1)] = idx.reshape(EPAD // 16, 16).T
            shard = src_pad[r] // GSPLIT  # true sub-partition in [0,16)
            m = np.zeros((16, EPAD), np.float32)
            m[shard, np.arange(EPAD)] = eval_pad[r] * used[r]
            gmask[q, 16 * g:16 * (g + 1)] = m

    ones32 = np.zeros((4, P, R), np.float32)
    for q in range(4):
        for g in range(8):
            ones32[q, 16 * g:16 * (g + 1), 8 * q + g] = 1

    return {
        "hvg": hvg, "gidx": gidx, "gmask": gmask,
        "Sdr": S, "Wdr": W,
        "hemb": hemb, "pemb": pemb,
        "ones32": ones32,
    }


def _build_host_inputs(entity_embedding, head_vector, head_emb, pred_emb,
                       edge_val, edge_src, edge_dst):
    entity_pad = np.zeros((NPAD, D), np.float32)
    entity_pad[:N] = entity_embedding
    # proj moving operand, grouped 8 n-tiles per DMA:
    # entNg[g, p, k*512 + dd] = entity[(8g+k)*128 + p, dd]
    npadg = ((T + 7) // 8) * 8 * P                  # 50176
    en_t = np.zeros((npadg // P, P, D), np.float32)
    en_t[:T] = entity_pad.reshape(T, P, D)
    entN = np.ascontiguousarray(
        en_t.reshape(-1, 8, P, D).transpose(0, 2, 1, 3).reshape(-1, P, 8 * D)
    ).astype(bf16)                                  # [49, 128, 4096]
    # score stationary operand, grouped NWIN n-tiles per DMA:
    # entTg[g, p, k*512 + dk*128 + j] = entity[(NWIN*g+k)*128 + j, dk*128 + p]
    ngrp = (T + NWIN - 1) // NWIN                   # 31
    et_t = np.zeros((ngrp * NWIN, P, D), np.float32)
    et_t[:T] = entity_pad.reshape(T, P, D)          # [tile, j, d]
    # -> [g, k, j, dk, p] -> [g, p, k, dk, j]
    et5 = et_t.reshape(ngrp, NWIN, P, 4, P).transpose(0, 4, 1, 3, 2)
    entT = np.ascontiguousarray(et5.reshape(ngrp, P, NWIN * D)).astype(bf16)
    ident = np.eye(P, dtype=np.float32)
    identb = np.eye(P, dtype=bf16)
    ones128 = np.ones((P, 1), np.float32)
    padm = np.zeros((P, 1), np.float32)
    padm[N - (T - 1) * P:] = -1e30   # partitions >= 80 in the last n-tile

    in_maps = []
    for core in range(NCORES):
        m = _prep_core(core, entity_pad, head_vector, head_emb, pred_emb,
                       edge_val, edge_src, edge_dst)
        m["entT"] = entT
        m["entN"] = entN
        m["ident"] = ident
        m["identb"] = identb
        m["ones128"] = ones128
        m["padm"] = padm
        in_maps.append(m)
    return in_maps


# ---------------------------------------------------------------------------
# Bass program
# ---------------------------------------------------------------------------

def build_program():
    from contextlib import ExitStack
    import concourse.bass as bass
    import concourse.tile as tile
    from concourse import bacc, mybir
    from concourse.alu_op_type import AluOpType as op
    import bass_rust

    dt = mybir.dt
    f32, b16, u16 = dt.float32, dt.bfloat16, dt.uint16
    Exp = bass_rust.ActivationFunctionType.Exp

    nc = bacc.Bacc("TRN2", target_bir_lowering=False, debug=False,
                   num_devices=NCORES)

    def din(name, shape, dtype):
        return nc.dram_tensor(name, list(shape), dtype, kind="ExternalInput").ap()

    NGRP = (T + NWIN - 1) // NWIN
    NPGRP = (T + 7) // 8
    entT = din("entT", (NGRP, P, NWIN * D), b16)
    entN = din("entN", (NPGRP, P, 8 * D), b16)
    hvg = din("hvg", (4, P, GSPLIT), f32)
    gidx = din("gidx", (4, P, EPAD // 16), u16)
    gmask = din("gmask", (4, P, EPAD), f32)
    Sdr = din("Sdr", (R, P, ECHUNKS * P), b16)
    Wdr = din("Wdr", (R, P, ECHUNKS * 32), b16)
    hemb = din("hemb", (R, D), f32)
    pemb = din("pemb", (R, D), f32)
    ones32 = din("ones32", (4, P, R), f32)
    ident = din("ident", (P, P), f32)
    identb = din("identb", (P, P), b16)
    ones128 = din("ones128", (P, 1), f32)
    padm = din("padm", (P, 1), f32)

    out = nc.dram_tensor("out", [BLOC, D], f32, kind="ExternalOutput").ap()

    with tile.TileContext(nc) as tc, ExitStack() as ctx:
        ctx.enter_context(nc.allow_low_precision(
            reason="bf16 storage is deliberate; reductions accumulate f32"))
        const = ctx.enter_context(tc.tile_pool(name="const", bufs=1))
        big = ctx.enter_context(tc.tile_pool(name="big", bufs=1))
        small = ctx.enter_context(tc.tile_pool(name="small", bufs=1))
        gat = ctx.enter_context(tc.tile_pool(name="gat", bufs=1))
        gat2 = ctx.enter_context(tc.tile_pool(name="gat2", bufs=2))
        entp = ctx.enter_context(tc.tile_pool(name="entp", bufs=2))
        srow = ctx.enter_context(tc.tile_pool(name="srow", bufs=1))
        srow2 = ctx.enter_context(tc.tile_pool(name="srow2", bufs=2))
        wmp = ctx.enter_context(tc.tile_pool(name="wmp", bufs=1))
        ps_score = ctx.enter_context(tc.tile_pool(name="ps_score", bufs=2, space="PSUM"))
        ps_sym = ctx.enter_context(tc.tile_pool(name="ps_sym", bufs=2, space="PSUM"))
        ps_tr = ctx.enter_context(tc.tile_pool(name="ps_tr", bufs=1, space="PSUM"))
        ps_trb = ctx.enter_context(tc.tile_pool(name="ps_trb", bufs=1, space="PSUM"))
        ps_proj = ctx.enter_context(tc.tile_pool(name="ps_proj", bufs=1, space="PSUM"))

        # ---- constants
        ident_sb = const.tile([P, P], f32)
        nc.sync.dma_start(ident_sb[:], ident[:])
        identb_sb = const.tile([P, P], b16)
        nc.sync.dma_start(identb_sb[:], identb[:])
        ones32_sb = const.tile([P, 4, R], f32)
        nc.sync.dma_start(ones32_sb[:], ones32[:].rearrange("q p r -> p q r"))
        ones128_sb = const.tile([P, 1], f32)
        nc.sync.dma_start(ones128_sb[:], ones128[:])
        padm_sb = const.tile([P, 1], f32)
        nc.sync.dma_start(padm_sb[:], padm[:])

        # ---- tail = head + pred -> transpose to [128d, dk*32 + r] bf16
        tail_f = small.tile([R, D], f32, tag="tailf")
        h_t = small.tile([R, D], f32, tag="hp")
        p_t = small.tile([R, D], f32, tag="hp2")
        nc.sync.dma_start(h_t[:], hemb[:])
        nc.sync.dma_start(p_t[:], pemb[:])
        nc.vector.tensor_tensor(out=tail_f[:], in0=h_t[:], in1=p_t[:], op=op.add)
        tailT = const.tile([P, 4 * R], b16)
        for dk in range(4):
            pt = ps_tr.tile([P, 512], f32, tag="tr")
            nc.tensor.transpose(out=pt[:, :R], in_=tail_f[:, dk * P:(dk + 1) * P],
                                identity=ident_sb[:R, :R])
            nc.vector.tensor_copy(out=tailT[:, dk * R:(dk + 1) * R], in_=pt[:, :R])

        # ---- score: psum [128, 13*32] bank-packed; score3 [128, T, R] bf16
        score3 = big.tile([P, T, R], b16, tag="bigA")
        for grp in range(NGRP):
            t0 = grp * NWIN
            cnt = min(NWIN, T - t0)
            et = entp.tile([P, NWIN, 4, P], b16, tag="et")
            nc.sync.dma_start(et[:], entT[grp])
            pss = ps_score.tile([P, TPAD], f32, tag="ps_s")
            for k in range(cnt):
                for dk in range(4):
                    nc.tensor.matmul(out=pss[:, 32 * k:32 * k + 32],
                                     lhsT=et[:, k, dk, :],
                                     rhs=tailT[:, dk * R:(dk + 1) * R],
                                     start=(dk == 0), stop=(dk == 3))
            nc.vector.tensor_copy(out=score3[:, t0:t0 + cnt, :],
                                  in_=pss[:, :32 * cnt])

        # mask pad entities (n >= 50000) before softmax max
        nc.vector.tensor_scalar(out=score3[:, T - 1:T, :],
                                in0=score3[:, T - 1:T, :],
                                scalar1=padm_sb[:], scalar2=None, op0=op.add)

        # ---- gather: msg[r, j] = edge_val * hv[src]  (gmask carries edge_val)
        msg_sb = const.tile([R, EPAD], b16)
        nc.vector.memset(msg_sb[:], 0)
        for q in range(4):
            hv_t = gat.tile([P, GSPLIT], f32, tag="hv")
            nc.sync.dma_start(hv_t[:], hvg[q])
            gi_t = gat2.tile([P, EPAD // 16], u16, tag="gi")
            nc.sync.dma_start(gi_t[:], gidx[q])
            gm_t = gat.tile([P, EPAD], f32, tag="gm")
            nc.sync.dma_start(gm_t[:], gmask[q])
            cand = gat.tile([P, EPAD], f32, tag="cand")
            for w in range(NWIN):
                nc.gpsimd.indirect_copy(
                    out=cand[:, 512 * w:512 * (w + 1)], data=hv_t[:],
                    idxs=gi_t[:, 32 * w:32 * (w + 1)],
                    i_know_ap_gather_is_preferred=True)
            nc.vector.tensor_tensor(out=cand[:], in0=cand[:], in1=gm_t[:],
                                    op=op.mult)
            for ch in range(NWIN):
                pg = ps_tr.tile([P, 512], f32, tag="tr")
                nc.tensor.matmul(out=pg[:R, :], lhsT=ones32_sb[:, q, :],
                                 rhs=cand[:, 512 * ch:512 * (ch + 1)],
                                 start=True, stop=True)
                nc.vector.tensor_tensor(
                    out=msg_sb[:, 512 * ch:512 * (ch + 1)],
                    in0=msg_sb[:, 512 * ch:512 * (ch + 1)],
                    in1=pg[:R, :], op=op.add)

        # ---- msgT [128, c*32 + r] bf16
        msgT = const.tile([P, ECHUNKS * R], b16)
        for c in range(ECHUNKS):
            pt = ps_trb.tile([P, 512], b16, tag="trb")
            nc.tensor.transpose(out=pt[:, :R], in_=msg_sb[:, c * P:(c + 1) * P],
                                identity=identb_sb[:R, :R])
            nc.vector.tensor_copy(out=msgT[:, c * R:(c + 1) * R], in_=pt[:, :R])
        msgT3 = msgT[:].rearrange("p (c r) -> p c r", r=R)

        # ---- scatter: per row, 52 one-hot matmuls into psum [128, 416]
        sym3 = big.tile([P, T, R], b16, tag="bigB")
        for r in range(R):
            st = srow.tile([P, ECHUNKS, P], b16, tag="srow")
            nc.sync.dma_start(st[:], Sdr[r])
            wt = srow2.tile([P, ECHUNKS, 32], b16, tag="wrow")
            nc.sync.dma_start(wt[:], Wdr[r])
            wm = wmp.tile([P, ECHUNKS, 32], b16, tag="wmsg")
            m_ap = msgT3[:, :, r:r + 1].to_broadcast([P, ECHUNKS, 32])
            nc.vector.tensor_tensor(out=wm[:], in0=wt[:], in1=m_ap, op=op.mult)
            psy = ps_sym.tile([P, TPAD], f32, tag="ps_y")
            for c in range(ECHUNKS):
                w, k = divmod(c, WCHUNK)
                nc.tensor.matmul(out=psy[:, 32 * w:32 * w + 32],
                                 lhsT=st[:, c, :],
                                 rhs=wm[:, c, :],
                                 start=(k == 0), stop=(k == WCHUNK - 1))
            nc.vector.tensor_copy(out=sym3[:, :, r:r + 1], in_=psy[:, :T])

        # ---- helpers: cross-partition reduce of [128, nr] f32 + bf16 re-broadcast
        def col_reduce_bcast(x_rc, nr, op_red, post, tagp):
            pt = ps_tr.tile([P, 512], f32, tag="tr")
            nc.tensor.transpose(out=pt[:nr, :P], in_=x_rc, identity=ident_sb[:])
            v = small.tile([nr, 1], f32, tag="v" + tagp)
            nc.vector.tensor_reduce(out=v[:], in_=pt[:nr, :P],
                                    axis=mybir.AxisListType.X, op=op_red)
            v2 = small.tile([nr, 1], b16, tag="w" + tagp)
            post(v, v2)
            pb = ps_trb.tile([P, 512], b16, tag="trb")
            vb = v2[:].to_broadcast([nr, P])
            nc.tensor.transpose(out=pb[:, :nr], in_=vb, identity=identb_sb[:nr, :nr])
            return pb

        def bcast_ap(ps_tile, nr):
            a = ps_tile[:, :nr]
            return bass.AP(a.tensor, a.offset, [list(a.ap[0]), [0, T], list(a.ap[1])])

        sc_flat = score3[:].rearrange("p t r -> p (t r)")
        sy_flat = sym3[:].rearrange("p t r -> p (t r)")
        sc_rt = score3[:].rearrange("p t r -> p r t")
        sy_rt = sym3[:].rearrange("p t r -> p r t")

        # ---- softmax(score)
        m1 = small.tile([P, R], f32, tag="m1")
        nc.vector.tensor_reduce(out=m1[:], in_=sc_rt, axis=mybir.AxisListType.X,
                                op=op.max)
        mB = col_reduce_bcast(m1[:], R, op.max,
                              lambda v, v2: nc.vector.tensor_copy(out=v2[:], in_=v[:]),
                              "m")
        nc.vector.tensor_tensor(out=sc_flat, in0=sc_flat, in1=bcast_ap(mB, R),
                                op=op.subtract)
        nc.scalar.activation(out=sc_flat, in_=sc_flat, func=Exp)
        s1 = small.tile([P, R], f32, tag="s1")
        nc.vector.tensor_reduce(out=s1[:], in_=sc_rt, axis=mybir.AxisListType.X,
                                op=op.add)

        def recip_post(v, v2):
            nc.vector.reciprocal(out=v2[:], in_=v[:])

        def clipmax_recip(v, v2):
            nc.vector.tensor_scalar(out=v[:], in0=v[:], scalar1=float(CLIP),
                                    scalar2=None, op0=op.max)
            nc.vector.reciprocal(out=v2[:], in_=v[:])

        sB = col_reduce_bcast(s1[:], R, op.add, recip_post, "s")
        nc.vector.tensor_tensor(out=sc_flat, in0=sc_flat, in1=bcast_ap(sB, R),
                                op=op.mult)

        # ---- clip_norm(symbolic)
        nc.vector.scalar_tensor_tensor(out=sy_flat, in0=sy_flat, scalar=CLIP,
                                       in1=sy_flat, op0=op.is_ge, op1=op.mult)
        y1 = small.tile([P, R], f32, tag="y1")
        nc.vector.tensor_reduce(out=y1[:], in_=sy_rt, axis=mybir.AxisListType.X,
                                op=op.add)
        yB = col_reduce_bcast(y1[:], R, op.add, clipmax_recip, "y")
        nc.vector.tensor_tensor(out=sy_flat, in0=sy_flat, in1=bcast_ap(yB, R),
                                op=op.mult)

        # ---- enhanced = clip_norm(sym_n + neural)
        nc.vector.tensor_tensor(out=sy_flat, in0=sy_flat, in1=sc_flat, op=op.add)
        nc.vector.scalar_tensor_tensor(out=sy_flat, in0=sy_flat, scalar=CLIP,
                                       in1=sy_flat, op0=op.is_ge, op1=op.mult)
        e1 = small.tile([P, R], f32, tag="e1")
        nc.vector.tensor_reduce(out=e1[:], in_=sy_rt, axis=mybir.AxisListType.X,
                                op=op.add)
        eB = col_reduce_bcast(e1[:], R, op.add, clipmax_recip, "e")
        nc.vector.tensor_tensor(out=sy_flat, in0=sy_flat, in1=bcast_ap(eB, R),
                                op=op.mult)

        # ---- agg = clip-mask(enh[a0] * enh[a1]); normalization deferred
        agg = big.tile([P, T, BLOC], f32, tag="bigA")
        nc.vector.tensor_tensor(out=agg[:], in0=sym3[:, :, :BLOC],
                                in1=sym3[:, :, BLOC:], op=op.mult)
        ag_flat = agg[:].rearrange("p t r -> p (t r)")
        nc.vector.scalar_tensor_tensor(out=ag_flat, in0=ag_flat, scalar=CLIP,
                                       in1=ag_flat, op0=op.is_ge, op1=op.mult)
        ag_rt = agg[:].rearrange("p t r -> p r t")
        g1 = small.tile([P, BLOC], f32, tag="g1")
        nc.vector.tensor_reduce(out=g1[:], in_=ag_rt, axis=mybir.AxisListType.X,
                                op=op.add)
        pd = ps_tr.tile([P, 512], f32, tag="tr")
        nc.tensor.matmul(out=pd[:BLOC, :1], lhsT=g1[:], rhs=ones128_sb[:],
                         start=True, stop=True)
        dclip = small.tile([BLOC, 1], f32, tag="dclip")
        nc.vector.tensor_scalar(out=dclip[:], in0=pd[:BLOC, :1],
                                scalar1=float(CLIP), scalar2=None, op0=op.max)
        drcp = small.tile([BLOC, 1], f32, tag="drcp")
        nc.vector.reciprocal(out=drcp[:], in_=dclip[:])

        # ---- projection: out = (agg_masked @ entity) * drcp
        agg_b = big.tile([P, T, BLOC], b16, tag="bigB")
        nc.vector.tensor_copy(out=agg_b[:], in_=agg[:])
        pso = ps_proj.tile([BLOC, D], f32)
        for g in range(NPGRP):
            en = entp.tile([P, 8, D], b16, tag="en")
            nc.sync.dma_start(en[:], entN[g])
            for k in range(8):
                nt = 8 * g + k
                if nt >= T:
                    break
                nc.tensor.matmul(out=pso[:], lhsT=agg_b[:, nt, :],
                                 rhs=en[:, k, :],
                                 start=(nt == 0), stop=(nt == T - 1))
        out_sb = small.tile([BLOC, D], f32, tag="outsb")
        nc.vector.tensor_scalar(out=out_sb[:], in0=pso[:], scalar1=drcp[:],
                                scalar2=None, op0=op.mult)
        nc.sync.dma_start(out[:], out_sb[:])

    nc.compile()
    return nc


_PROGRAM = None


def kernel(entity_embedding, head_vector, head_emb, pred_emb,
           edge_val, edge_src, edge_dst):
    global _PROGRAM
    from concourse.bass_utils import run_bass_kernel_spmd

    in_maps = _build_host_inputs(entity_embedding, head_vector,
                                 head_emb, pred_emb,
                                 edge_val, edge_src, edge_dst)
    if _PROGRAM is None:
        _PROGRAM = build_program()
    res = run_bass_kernel_spmd(_PROGRAM, in_maps, list(range(NCORES)))
    out = np.empty((B, D), np.float32)
    for c in range(NCORES):
        out[c * BLOC:(c + 1) * BLOC] = res.results[c]["out"]
    return out


if __name__ == "__main__":
    import reference
    inputs = {k: np.asarray(v) for k, v in reference.setup_inputs().items()}
    got = kernel(**inputs)
    want = np.asarray(reference.reference(**inputs))
    err = np.abs(got - want).max() / np.abs(want).max()
    print("Relative error:", err)



# revision 5
# speedup vs baseline: 2.7786x; 2.7786x over previous
"""Trainium2 Bass kernel for NeuralSymbolicMP layer (gnn_message_passing).

Batch-sharded over B across 8 NeuronCores; each core handles 32 (atomic,
batch) rows x all N entities.

Symbolic message passing runs as a GpSimd local_scatter permutation network:
  hv (src-partition layout) --local_scatter--> staging --PE transpose-->
  B (dst-partition layout) --local_scatter--> M (per-edge, dst-ordered)
  --x val--> --local_scatter--> dense sym3 [128, T, R]
with duplicate-src edges resolved by sibling-read passes over M and
duplicate-dst edges by pair slots + spill layers.

Neural side (score matmul, softmax, clip-norm chain, projection) is a dense
PE/DVE pipeline over entity tiles streamed from HBM in bf16.
"""

import numpy as np
import ml_dtypes

A, B, N, D, E = 2, 128, 50000, 512, 4096
CLIP = 1e-14

NCORES = 8
BLOC = B // NCORES          # 16 batch per core
R = A * BLOC                # 32 rows per core
P = 128
T = 391                     # n tiles: 391*128 = 50048
NPAD = T * P                # 50048
NWIN = 13                   # score psum windows of 32 tiles
TPAD = NWIN * 32            # 416

# symbolic permutation network constants
NG = 4                      # row groups for staging
GR = R // NG                # 8 rows per group
C1 = 13                     # staging chunks per group
TCH = 56                    # t-tiles per M chunk
NCH = 7                     # M chunks
SING = 224                  # singles per (p, chunk)
PAIR = 16                   # pair slots per (p, chunk)
MSTRIDE = SING + 2 * PAIR   # 256
C2 = 6                      # dupe staging chunks
M2CAP = 24                  # M2 capacity per (p, chunk)
NDUPE = 3                   # dupe passes
SPSTRIDE = PAIR + NDUPE * M2CAP  # 88
NSPILL = 3                  # spill layers
SYM_LEN = T * R             # 12512
BLEN = NG * C1 * P          # 6656

bf16 = ml_dtypes.bfloat16

# spill build plan: set of (layer, chunk) with any entry on any core.
# populated by _build_host_inputs before build_program is called.
_SPILL_PLAN = [(L, c) for L in range(NSPILL) for c in range(NCH)]


def _chunk_elems(ch):
    t0, t1 = ch * TCH, min((ch + 1) * TCH, T)
    return (t1 - t0) * R


def _cumcount(keys, sel):
    """For each selected element, its 0-based rank among same-key elements."""
    idxs = np.nonzero(sel)[0]
    if idxs.size == 0:
        return idxs, np.zeros(0, np.int64)
    kk = keys[idxs]
    o = np.argsort(kk, kind="stable")
    k = kk[o]
    n = np.ones(idxs.size, bool)
    n[1:] = k[1:] != k[:-1]
    g = np.cumsum(n) - 1
    f = np.zeros(g.max() + 1, np.int64)
    f[g[n]] = np.nonzero(n)[0]
    cnt_sorted = np.arange(idxs.size) - f[g]
    out = np.empty(idxs.size, np.int64)
    out[o] = cnt_sorted
    return idxs, out


def _prep_symbolic(hv_core, src, dst, val):
    """hv_core [R, NPAD] f32, src/dst/val [R, E]. Returns device arrays +
    spill plan entries."""
    r = np.repeat(np.arange(R), E)
    s = src.ravel().astype(np.int64)
    d = dst.ravel().astype(np.int64)
    v = val.ravel().astype(np.float32)
    NE = r.size

    p_s, t_s = s % P, s // P
    p_d, t_d = d % P, d // P
    g = r // GR
    ch = t_d // TCH

    # src-group ranks
    key_s = r * NPAD + s
    o = np.argsort(key_s, kind="stable")
    ks = key_s[o]
    newg = np.ones(NE, bool)
    newg[1:] = ks[1:] != ks[:-1]
    gid = np.cumsum(newg) - 1
    first = np.zeros(gid.max() + 1, np.int64)
    first[gid[newg]] = np.nonzero(newg)[0]
    srank = np.empty(NE, np.int64)
    srank[o] = np.arange(NE) - first[gid]
    assert srank.max() <= NDUPE, f"src dupe depth {srank.max()}"
    main = srank == 0
    sibling = np.empty(NE, np.int64)
    sibling[o] = o[first[gid]]

    # staging chunks for main edges
    mi, c1 = _cumcount((g * P + p_s) * P + p_d, main)
    assert c1.max() < C1, f"C1 overflow {c1.max() + 1}"

    # dst groups: mains first
    key_d = r * NPAD + d
    od = np.argsort(key_d * 8 + np.minimum(srank, 7), kind="stable")
    kd = key_d[od]
    ngd = np.ones(NE, bool)
    ngd[1:] = kd[1:] != kd[:-1]
    gidd = np.cumsum(ngd) - 1
    fd = np.zeros(gidd.max() + 1, np.int64)
    fd[gidd[ngd]] = np.nonzero(ngd)[0]
    drank = np.empty(NE, np.int64)
    drank[od] = np.arange(NE) - fd[gidd]
    gmain = np.zeros(gidd.max() + 1, np.int64)
    np.add.at(gmain, gidd, main[od].astype(np.int64))
    qmain = np.empty(NE, np.int64)
    qmain[od] = gmain[gidd]

    is_single = main & (qmain == 1)
    is_pairm = main & (qmain >= 2)
    cellM = p_d * NCH + ch

    si, scnt = _cumcount(cellM, is_single)
    assert scnt.size == 0 or scnt.max() < SING, f"SING overflow {scnt.max()+1}"
    pair_lead = is_pairm & (drank % 2 == 0)
    pi, pcnt = _cumcount(cellM, pair_lead)
    assert pcnt.size == 0 or pcnt.max() < PAIR, f"PAIR overflow {pcnt.max()+1}"
    pairslot = {(key_d[e], drank[e] // 2): c for e, c in zip(pi, pcnt)}

    Mslot = np.full(NE, -1, np.int64)
    Mslot[si] = ch[si] * MSTRIDE + scnt
    pm = np.nonzero(is_pairm)[0]
    for e in pm:
        c = pairslot[(key_d[e], drank[e] // 2)]
        Mslot[e] = ch[e] * MSTRIDE + SING + 2 * c + (drank[e] % 2)

    m2slot = np.full(NE, -1, np.int64)
    m2pass = np.full(NE, -1, np.int64)
    for k in range(NDUPE):
        ki, kcnt = _cumcount(cellM, srank == k + 1)
        assert kcnt.size == 0 or kcnt.max() < M2CAP, f"M2 overflow {kcnt.max()+1}"
        m2slot[ki] = ch[ki] * M2CAP + kcnt
        m2pass[ki] = k

    c2 = np.full(NE, -1, np.int64)
    for k in range(NDUPE):
        ki, cko = _cumcount(p_d[sibling] * P + p_d, srank == k + 1)
        assert cko.size == 0 or cko.max() < C2, f"C2 overflow {cko.max()+1}"
        c2[ki] = cko

    # spill layers
    sym_of = t_d * R + r
    spill_ctr = {}
    pair_spill = {}
    for e in pi:
        kk = (p_d[e], sym_of[e])
        L = spill_ctr.get(kk, 0)
        assert L < NSPILL, "spill overflow"
        spill_ctr[kk] = L + 1
        pair_spill[(key_d[e], drank[e] // 2)] = L
    m2_spill = np.full(NE, -1, np.int64)
    for e in np.nonzero(srank >= 1)[0]:
        kk = (p_d[e], sym_of[e])
        L = spill_ctr.get(kk, 0)
        assert L < NSPILL, "spill overflow"
        spill_ctr[kk] = L + 1
        m2_spill[e] = L

    # ---- device arrays
    arr = {}
    idx1 = np.full((NG, P, GR * T), -1, np.int16)
    ma = np.nonzero(main)[0]
    idx1[g[ma], p_s[ma], (r[ma] % GR) * T + t_s[ma]] = \
        (c1 * P + p_d[mi]).astype(np.int16)
    arr["idx1"] = idx1

    idx2 = np.full((P, BLEN), -1, np.int16)
    idx2[p_d[mi], (g[mi] * C1 + c1) * P + p_s[mi]] = Mslot[mi].astype(np.int16)
    arr["idx2"] = idx2

    valM = np.zeros((P, NCH * MSTRIDE), np.float32)
    valM[p_d[mi], Mslot[mi]] = v[mi]
    arr["valM"] = valM.astype(bf16)

    idxd = np.full((NDUPE, P, NCH * MSTRIDE), -1, np.int16)
    idx2d = np.full((NDUPE, P, C2 * P), -1, np.int16)
    valM2 = np.zeros((NDUPE, P, NCH * M2CAP), np.float32)
    for k in range(NDUPE):
        ki = np.nonzero(srank == k + 1)[0]
        sib = sibling[ki]
        idxd[k, p_d[sib], Mslot[sib]] = (c2[ki] * P + p_d[ki]).astype(np.int16)
        idx2d[k, p_d[ki], c2[ki] * P + p_d[sib]] = m2slot[ki].astype(np.int16)
        valM2[k, p_d[ki], m2slot[ki]] = v[ki]
    arr["idxd"] = idxd
    arr["idx2d"] = idx2d
    arr["valM2"] = valM2.astype(bf16)

    idx3 = np.full((P, NCH * SING), -1, np.int16)
    idx3[p_d[si], ch[si] * SING + scnt] = \
        ((t_d[si] - ch[si] * TCH) * R + r[si]).astype(np.int16)
    arr["idx3"] = idx3

    idxsp = np.full((NSPILL, P, NCH * SPSTRIDE), -1, np.int16)
    plan = set()
    for e in pi:
        c = pairslot[(key_d[e], drank[e] // 2)]
        L = pair_spill[(key_d[e], drank[e] // 2)]
        idxsp[L, p_d[e], ch[e] * SPSTRIDE + c] = \
            (t_d[e] - ch[e] * TCH) * R + r[e]
        plan.add((L, int(ch[e])))
    for e in np.nonzero(srank >= 1)[0]:
        k = m2pass[e]
        L = m2_spill[e]
        rel = m2slot[e] - ch[e] * M2CAP
        idxsp[L, p_d[e], ch[e] * SPSTRIDE + PAIR + k * M2CAP + rel] = \
            (t_d[e] - ch[e] * TCH) * R + r[e]
        plan.add((int(L), int(ch[e])))
    arr["idxsp"] = idxsp

    arr["hv_sb"] = np.ascontiguousarray(
        hv_core.reshape(R, T, P).transpose(2, 0, 1).reshape(P, R * T)
    ).astype(bf16)
    return arr, plan


def _prep_core(core, head_vector, head_emb, pred_emb, edge_val, edge_src,
               edge_dst):
    b0 = core * BLOC
    hv = np.zeros((R, NPAD), np.float32)
    ev = np.empty((R, E), np.float32)
    es = np.empty((R, E), np.int64)
    ed = np.empty((R, E), np.int64)
    hemb = np.empty((R, D), np.float32)
    pemb = np.empty((R, D), np.float32)
    for a in range(A):
        sl = slice(a * BLOC, (a + 1) * BLOC)
        hv[sl, :N] = head_vector[a, b0:b0 + BLOC]
        ev[sl] = edge_val[a, b0:b0 + BLOC]
        es[sl] = edge_src[a, b0:b0 + BLOC]
        ed[sl] = edge_dst[a, b0:b0 + BLOC]
        hemb[sl] = head_emb[a, b0:b0 + BLOC]
        pemb[sl] = pred_emb[a, b0:b0 + BLOC]

    arr, plan = _prep_symbolic(hv, es, ed, ev)
    arr["hemb"] = hemb
    arr["pemb"] = pemb
    return arr, plan


def _build_host_inputs(entity_embedding, head_vector, head_emb, pred_emb,
                       edge_val, edge_src, edge_dst):
    global _SPILL_PLAN
    entity_pad = np.zeros((NPAD, D), np.float32)
    entity_pad[:N] = entity_embedding
    # proj moving operand entN: [g, p, k*512+dd] = entity[(8g+k)*128+p, dd]
    npadg = ((T + 7) // 8) * 8 * P
    en_t = np.zeros((npadg // P, P, D), np.float32)
    en_t[:T] = entity_pad.reshape(T, P, D)
    entN = np.ascontiguousarray(
        en_t.reshape(-1, 8, P, D).transpose(0, 2, 1, 3).reshape(-1, P, 8 * D)
    ).astype(bf16)
    # score stationary entT: [g, p, k*512+dk*128+j] = entity[(13g+k)*128+j, dk*128+p]
    ngrp = (T + NWIN - 1) // NWIN
    et_t = np.zeros((ngrp * NWIN, P, D), np.float32)
    et_t[:T] = entity_pad.reshape(T, P, D)
    et5 = et_t.reshape(ngrp, NWIN, P, 4, P).transpose(0, 4, 1, 3, 2)
    entT = np.ascontiguousarray(et5.reshape(ngrp, P, NWIN * D)).astype(bf16)
    ident = np.eye(P, dtype=np.float32)
    identb = np.eye(P, dtype=bf16)
    ones128 = np.ones((P, 1), np.float32)
    padm = np.zeros((P, 1), np.float32)
    padm[N - (T - 1) * P:] = -1e30

    in_maps = []
    plan_all = set()
    for core in range(NCORES):
        m, plan = _prep_core(core, head_vector, head_emb, pred_emb,
                             edge_val, edge_src, edge_dst)
        plan_all |= plan
        m["entT"] = entT
        m["entN"] = entN
        m["ident"] = ident
        m["identb"] = identb
        m["ones128"] = ones128
        m["padm"] = padm
        in_maps.append(m)
    _SPILL_PLAN = sorted(plan_all)
    return in_maps


# ---------------------------------------------------------------------------
# Bass program
# ---------------------------------------------------------------------------

def build_program():
    from contextlib import ExitStack
    import concourse.bass as bass
    import concourse.tile as tile
    from concourse import bacc, mybir
    from concourse.alu_op_type import AluOpType as op
    import bass_rust

    dt = mybir.dt
    f32, b16, i16 = dt.float32, dt.bfloat16, dt.int16
    Exp = bass_rust.ActivationFunctionType.Exp

    nc = bacc.Bacc("TRN2", target_bir_lowering=False, debug=False,
                   num_devices=NCORES)

    def din(name, shape, dtype):
        return nc.dram_tensor(name, list(shape), dtype, kind="ExternalInput").ap()

    NGRP = (T + NWIN - 1) // NWIN
    NPGRP = (T + 7) // 8
    entT = din("entT", (NGRP, P, NWIN * D), b16)
    entN = din("entN", (NPGRP, P, 8 * D), b16)
    hemb = din("hemb", (R, D), f32)
    pemb = din("pemb", (R, D), f32)
    ident = din("ident", (P, P), f32)
    identb = din("identb", (P, P), b16)
    ones128 = din("ones128", (P, 1), f32)
    padm = din("padm", (P, 1), f32)
    hv_sb_d = din("hv_sb", (P, R * T), b16)
    idx1_d = din("idx1", (NG, P, GR * T), i16)
    idx2_d = din("idx2", (P, BLEN), i16)
    valM_d = din("valM", (P, NCH * MSTRIDE), b16)
    idxd_d = din("idxd", (NDUPE, P, NCH * MSTRIDE), i16)
    idx2d_d = din("idx2d", (NDUPE, P, C2 * P), i16)
    valM2_d = din("valM2", (NDUPE, P, NCH * M2CAP), b16)
    idx3_d = din("idx3", (P, NCH * SING), i16)
    idxsp_d = din("idxsp", (NSPILL, P, NCH * SPSTRIDE), i16)

    out = nc.dram_tensor("out", [BLOC, D], f32, kind="ExternalOutput").ap()

    with tile.TileContext(nc) as tc, ExitStack() as ctx:
        ctx.enter_context(nc.allow_low_precision(
            reason="bf16 storage is deliberate; reductions accumulate f32"))
        const = ctx.enter_context(tc.tile_pool(name="const", bufs=1))
        big = ctx.enter_context(tc.tile_pool(name="big", bufs=1))
        small = ctx.enter_context(tc.tile_pool(name="small", bufs=1))
        sym = ctx.enter_context(tc.tile_pool(name="sym", bufs=1))
        symi = ctx.enter_context(tc.tile_pool(name="symi", bufs=2))
        stgp = ctx.enter_context(tc.tile_pool(name="stgp", bufs=4))
        entp = ctx.enter_context(tc.tile_pool(name="entp", bufs=2))
        ps_score = ctx.enter_context(tc.tile_pool(name="ps_score", bufs=2, space="PSUM"))
        ps_tr = ctx.enter_context(tc.tile_pool(name="ps_tr", bufs=1, space="PSUM"))
        ps_trb = ctx.enter_context(tc.tile_pool(name="ps_trb", bufs=2, space="PSUM"))
        ps_proj = ctx.enter_context(tc.tile_pool(name="ps_proj", bufs=1, space="PSUM"))

        # ---- constants
        ident_sb = const.tile([P, P], f32)
        nc.sync.dma_start(ident_sb[:], ident[:])
        identb_sb = const.tile([P, P], b16)
        nc.sync.dma_start(identb_sb[:], identb[:])
        ones128_sb = const.tile([P, 1], f32)
        nc.sync.dma_start(ones128_sb[:], ones128[:])
        padm_sb = const.tile([P, 1], f32)
        nc.sync.dma_start(padm_sb[:], padm[:])

        # ---- symbolic small input DMAs
        idx2_t = sym.tile([P, BLEN], i16, tag="idx2")
        nc.sync.dma_start(idx2_t[:], idx2_d[:])
        valM_t = sym.tile([P, NCH * MSTRIDE], b16, tag="valM")
        nc.sync.dma_start(valM_t[:], valM_d[:])
        idx2d_t = sym.tile([P, NDUPE, C2 * P], i16, tag="idx2d")
        nc.sync.dma_start(idx2d_t[:], idx2d_d[:].rearrange("k p n -> p k n"))
        valM2_t = sym.tile([P, NDUPE, NCH * M2CAP], b16, tag="valM2")
        nc.sync.dma_start(valM2_t[:], valM2_d[:].rearrange("k p n -> p k n"))
        idx3_t = sym.tile([P, NCH * SING], i16, tag="idx3")
        nc.sync.dma_start(idx3_t[:], idx3_d[:])
        idxsp_t = sym.tile([P, NSPILL, NCH * SPSTRIDE], i16, tag="idxsp")
        nc.sync.dma_start(idxsp_t[:], idxsp_d[:].rearrange("k p n -> p k n"))

        # ---- tail = head + pred -> tailT [128d, dk*32+r] bf16
        tail_f = small.tile([R, D], f32, tag="tailf")
        h_t = small.tile([R, D], f32, tag="hp")
        p_t = small.tile([R, D], f32, tag="hp2")
        nc.sync.dma_start(h_t[:], hemb[:])
        nc.sync.dma_start(p_t[:], pemb[:])
        nc.vector.tensor_tensor(out=tail_f[:], in0=h_t[:], in1=p_t[:], op=op.add)
        tailT = const.tile([P, 4 * R], b16)
        for dk in range(4):
            pt = ps_tr.tile([P, 512], f32, tag="tr")
            nc.tensor.transpose(out=pt[:, :R], in_=tail_f[:, dk * P:(dk + 1) * P],
                                identity=ident_sb[:R, :R])
            nc.vector.tensor_copy(out=tailT[:, dk * R:(dk + 1) * R], in_=pt[:, :R])

        # ---- symbolic stage 1: 4 x (dma hv slice + local_scatter)
        stg = []
        for g in range(NG):
            hvg = symi.tile([P, GR * T], b16, tag="hvg")
            nc.sync.dma_start(hvg[:], hv_sb_d[:, g * GR * T:(g + 1) * GR * T])
            i1 = symi.tile([P, GR * T], i16, tag="idx1")
            nc.sync.dma_start(i1[:], idx1_d[g])
            st = stgp.tile([P, C1 * P], b16, tag="stg")
            nc.gpsimd.local_scatter(st[:], hvg[:], i1[:], channels=P,
                                    num_elems=C1 * P, num_idxs=GR * T)
            stg.append(st)

        Bt = sym.tile([P, BLEN], b16, tag="B")
        M_t = sym.tile([P, NCH * MSTRIDE], b16, tag="M")
        M2 = []
        spill = sym.tile([P, NCH, SPSTRIDE], b16, tag="spill")
        sym3 = big.tile([P, T, R], b16, tag="bigB")
        sym3f = sym3[:].rearrange("p t r -> p (t r)")

        def do_b_transposes():
            for g in range(NG):
                for c in range(C1):
                    pt = ps_trb.tile([P, 512], b16, tag="trb")
                    nc.tensor.transpose(out=pt[:, :P],
                                        in_=stg[g][:, c * P:(c + 1) * P],
                                        identity=identb_sb[:])
                    nc.vector.tensor_copy(
                        out=Bt[:, (g * C1 + c) * P:(g * C1 + c + 1) * P],
                        in_=pt[:, :P])
            nc.gpsimd.local_scatter(M_t[:], Bt[:], idx2_t[:], channels=P,
                                    num_elems=NCH * MSTRIDE, num_idxs=BLEN)

        def do_dupe_pass(k):
            idxd_t = symi.tile([P, NCH * MSTRIDE], i16, tag="idxd")
            nc.sync.dma_start(idxd_t[:], idxd_d[k])
            st2 = symi.tile([P, C2 * P], b16, tag="stg2")
            nc.gpsimd.local_scatter(st2[:], M_t[:], idxd_t[:], channels=P,
                                    num_elems=C2 * P, num_idxs=NCH * MSTRIDE)
            Bd = symi.tile([P, C2 * P], b16, tag="Bd")
            for c in range(C2):
                pt = ps_trb.tile([P, 512], b16, tag="trb")
                nc.tensor.transpose(out=pt[:, :P], in_=st2[:, c * P:(c + 1) * P],
                                    identity=identb_sb[:])
                nc.vector.tensor_copy(out=Bd[:, c * P:(c + 1) * P], in_=pt[:, :P])
            m2t = sym.tile([P, NCH * M2CAP], b16, tag=f"M2_{k}")
            nc.gpsimd.local_scatter(m2t[:], Bd[:], idx2d_t[:, k, :], channels=P,
                                    num_elems=NCH * M2CAP, num_idxs=C2 * P)
            M2.append(m2t)

        def do_symbolic_finish():
            # val multiply in place (dupe passes already read raw M)
            nc.vector.tensor_tensor(out=M_t[:], in0=M_t[:], in1=valM_t[:],
                                    op=op.mult)
            # pair reduce into spill[:, :, :PAIR]
            a = M_t[:]
            in0 = bass.AP(a.tensor, a.offset + SING,
                          [list(a.ap[0]), [MSTRIDE, NCH], [2, PAIR]])
            in1 = bass.AP(a.tensor, a.offset + SING + 1,
                          [list(a.ap[0]), [MSTRIDE, NCH], [2, PAIR]])
            nc.vector.tensor_tensor(out=spill[:, :, :PAIR], in0=in0, in1=in1,
                                    op=op.add)
            for k in range(NDUPE):
                m2v = M2[k][:].rearrange("p (ch s) -> p ch s", s=M2CAP)
                nc.vector.tensor_tensor(
                    out=m2v, in0=m2v,
                    in1=valM2_t[:, k, :].rearrange("p (ch s) -> p ch s", s=M2CAP),
                    op=op.mult)
                nc.vector.tensor_copy(
                    out=spill[:, :, PAIR + k * M2CAP:PAIR + (k + 1) * M2CAP],
                    in_=m2v)
            # 3: main singles into dense sym3
            for c in range(NCH):
                ne = _chunk_elems(c)
                nc.gpsimd.local_scatter(
                    sym3f[:, c * TCH * R: c * TCH * R + ne],
                    M_t[:, c * MSTRIDE:c * MSTRIDE + SING],
                    idx3_t[:, c * SING:(c + 1) * SING],
                    channels=P, num_elems=ne, num_idxs=SING)
            # spill layers
            for (L, c) in _SPILL_PLAN:
                ne = _chunk_elems(c)
                ssp = sym.tile([P, TCH * R], b16, tag="ssp")
                nc.gpsimd.local_scatter(
                    ssp[:, :ne], spill[:, c, :],
                    idxsp_t[:, L, c * SPSTRIDE:(c + 1) * SPSTRIDE],
                    channels=P, num_elems=ne, num_idxs=SPSTRIDE)
                nc.vector.tensor_tensor(
                    out=sym3f[:, c * TCH * R: c * TCH * R + ne],
                    in0=sym3f[:, c * TCH * R: c * TCH * R + ne],
                    in1=ssp[:, :ne], op=op.add)

        hooks = {10: do_b_transposes,
                 16: lambda: do_dupe_pass(0),
                 18: lambda: do_dupe_pass(1),
                 20: lambda: do_dupe_pass(2),
                 23: do_symbolic_finish}

        # ---- score: psum [128, 13*32] bank-packed; score3 [128, T, R] bf16
        score3 = big.tile([P, T, R], b16, tag="bigA")
        for grp in range(NGRP):
            if grp in hooks:
                hooks[grp]()
            t0 = grp * NWIN
            cnt = min(NWIN, T - t0)
            et = entp.tile([P, NWIN, 4, P], b16, tag="et")
            nc.sync.dma_start(et[:], entT[grp])
            pss = ps_score.tile([P, TPAD], f32, tag="ps_s")
            for k in range(cnt):
                for dk in range(4):
                    nc.tensor.matmul(out=pss[:, 32 * k:32 * k + 32],
                                     lhsT=et[:, k, dk, :],
                                     rhs=tailT[:, dk * R:(dk + 1) * R],
                                     start=(dk == 0), stop=(dk == 3))
            nc.vector.tensor_copy(out=score3[:, t0:t0 + cnt, :],
                                  in_=pss[:, :32 * cnt])

        # mask pad entities (n >= 50000) before softmax max
        nc.vector.tensor_scalar(out=score3[:, T - 1:T, :],
                                in0=score3[:, T - 1:T, :],
                                scalar1=padm_sb[:], scalar2=None, op0=op.add)

        # ---- helpers: cross-partition reduce of [128, nr] f32 + bf16 bcast
        def col_reduce_bcast(x_rc, nr, op_red, post, tagp):
            pt = ps_tr.tile([P, 512], f32, tag="tr")
            nc.tensor.transpose(out=pt[:nr, :P], in_=x_rc, identity=ident_sb[:])
            v = small.tile([nr, 1], f32, tag="v" + tagp)
            nc.vector.tensor_reduce(out=v[:], in_=pt[:nr, :P],
                                    axis=mybir.AxisListType.X, op=op_red)
            v2 = small.tile([nr, 1], b16, tag="w" + tagp)
            post(v, v2)
            pb = ps_trb.tile([P, 512], b16, tag="trb")
            vb = v2[:].to_broadcast([nr, P])
            nc.tensor.transpose(out=pb[:, :nr], in_=vb, identity=identb_sb[:nr, :nr])
            return pb

        def bcast_ap(ps_tile, nr):
            a = ps_tile[:, :nr]
            return bass.AP(a.tensor, a.offset,
                           [list(a.ap[0]), [0, T], list(a.ap[1])])

        sc_flat = score3[:].rearrange("p t r -> p (t r)")
        sy_flat = sym3[:].rearrange("p t r -> p (t r)")
        sc_rt = score3[:].rearrange("p t r -> p r t")
        sy_rt = sym3[:].rearrange("p t r -> p r t")

        # ---- softmax(score)
        m1 = small.tile([P, R], f32, tag="m1")
        nc.vector.tensor_reduce(out=m1[:], in_=sc_rt, axis=mybir.AxisListType.X,
                                op=op.max)
        mB = col_reduce_bcast(m1[:], R, op.max,
                              lambda v, v2: nc.vector.tensor_copy(out=v2[:], in_=v[:]),
                              "m")
        nc.vector.tensor_tensor(out=sc_flat, in0=sc_flat, in1=bcast_ap(mB, R),
                                op=op.subtract)
        nc.scalar.activation(out=sc_flat, in_=sc_flat, func=Exp)
        s1 = small.tile([P, R], f32, tag="s1")
        nc.vector.tensor_reduce(out=s1[:], in_=sc_rt, axis=mybir.AxisListType.X,
                                op=op.add)

        def recip_post(v, v2):
            nc.vector.reciprocal(out=v2[:], in_=v[:])

        def clipmax_recip(v, v2):
            nc.vector.tensor_scalar(out=v[:], in0=v[:], scalar1=float(CLIP),
                                    scalar2=None, op0=op.max)
            nc.vector.reciprocal(out=v2[:], in_=v[:])

        sB = col_reduce_bcast(s1[:], R, op.add, recip_post, "s")
        nc.vector.tensor_tensor(out=sc_flat, in0=sc_flat, in1=bcast_ap(sB, R),
                                op=op.mult)

        # ---- clip_norm(symbolic)
        nc.vector.scalar_tensor_tensor(out=sy_flat, in0=sy_flat, scalar=CLIP,
                                       in1=sy_flat, op0=op.is_ge, op1=op.mult)
        y1 = small.tile([P, R], f32, tag="y1")
        nc.vector.tensor_reduce(out=y1[:], in_=sy_rt, axis=mybir.AxisListType.X,
                                op=op.add)
        yB = col_reduce_bcast(y1[:], R, op.add, clipmax_recip, "y")
        nc.vector.tensor_tensor(out=sy_flat, in0=sy_flat, in1=bcast_ap(yB, R),
                                op=op.mult)

        # ---- enhanced = clip_norm(sym_n + neural)
        nc.vector.tensor_tensor(out=sy_flat, in0=sy_flat, in1=sc_flat, op=op.add)
        nc.vector.scalar_tensor_tensor(out=sy_flat, in0=sy_flat, scalar=CLIP,
                                       in1=sy_flat, op0=op.is_ge, op1=op.mult)
        e1 = small.tile([P, R], f32, tag="e1")
        nc.vector.tensor_reduce(out=e1[:], in_=sy_rt, axis=mybir.AxisListType.X,
                                op=op.add)
        eB = col_reduce_bcast(e1[:], R, op.add, clipmax_recip, "e")
        nc.vector.tensor_tensor(out=sy_flat, in0=sy_flat, in1=bcast_ap(eB, R),
                                op=op.mult)

        # ---- agg = clip-mask(enh[a0] * enh[a1]); normalization deferred
        agg = big.tile([P, T, BLOC], f32, tag="bigA")
        nc.vector.tensor_tensor(out=agg[:], in0=sym3[:, :, :BLOC],
                                in1=sym3[:, :, BLOC:], op=op.mult)
        ag_flat = agg[:].rearrange("p t r -> p (t r)")
        nc.vector.scalar_tensor_tensor(out=ag_flat, in0=ag_flat, scalar=CLIP,
                                       in1=ag_flat, op0=op.is_ge, op1=op.mult)
        ag_rt = agg[:].rearrange("p t r -> p r t")
        g1 = small.tile([P, BLOC], f32, tag="g1")
        nc.vector.tensor_reduce(out=g1[:], in_=ag_rt, axis=mybir.AxisListType.X,
                                op=op.add)
        pd = ps_tr.tile([P, 512], f32, tag="tr")
        nc.tensor.matmul(out=pd[:BLOC, :1], lhsT=g1[:], rhs=ones128_sb[:],
                         start=True, stop=True)
        dclip = small.tile([BLOC, 1], f32, tag="dclip")
        nc.vector.tensor_scalar(out=dclip[:], in0=pd[:BLOC, :1],
                                scalar1=float(CLIP), scalar2=None, op0=op.max)
        drcp = small.tile([BLOC, 1], f32, tag="drcp")
        nc.vector.reciprocal(out=drcp[:], in_=dclip[:])

        # ---- projection: out = (agg_masked @ entity) * drcp
        agg_b = big.tile([P, T, BLOC], b16, tag="bigB")
        nc.vector.tensor_copy(out=agg_b[:], in_=agg[:])
        pso = ps_proj.tile([BLOC, D], f32)
        for g in range(NPGRP):
            en = entp.tile([P, 8, D], b16, tag="en")
            nc.sync.dma_start(en[:], entN[g])
            for k in range(8):
                nt = 8 * g + k
                if nt >= T:
                    break
                nc.tensor.matmul(out=pso[:], lhsT=agg_b[:, nt, :],
                                 rhs=en[:, k, :],
                                 start=(nt == 0), stop=(nt == T - 1))
        out_sb = small.tile([BLOC, D], f32, tag="outsb")
        nc.vector.tensor_scalar(out=out_sb[:], in0=pso[:], scalar1=drcp[:],
                                scalar2=None, op0=op.mult)
        nc.sync.dma_start(out[:], out_sb[:])

    nc.compile()
    return nc


_PROGRAM = None


def kernel(entity_embedding, head_vector, head_emb, pred_emb,
           edge_val, edge_src, edge_dst):
    global _PROGRAM
    from concourse.bass_utils import run_bass_kernel_spmd

    in_maps = _build_host_inputs(entity_embedding, head_vector,
                                 head_emb, pred_emb,
                                 edge_val, edge_src, edge_dst)
    if _PROGRAM is None:
        _PROGRAM = build_program()
    res = run_bass_kernel_spmd(_PROGRAM, in_maps, list(range(NCORES)))
    out = np.empty((B, D), np.float32)
    for c in range(NCORES):
        out[c * BLOC:(c + 1) * BLOC] = res.results[c]["out"]
    return out


if __name__ == "__main__":
    import reference
    inputs = {k: np.asarray(v) for k, v in reference.setup_inputs().items()}
    got = kernel(**inputs)
    want = np.asarray(reference.reference(**inputs))
    err = np.abs(got - want).max() / np.abs(want).max()
    print("Relative error:", err)


# revision 6
# speedup vs baseline: 3.5584x; 1.2806x over previous
"""Trainium2 Bass kernel for NeuralSymbolicMP layer (gnn_message_passing).

Batch-sharded over B across 8 NeuronCores; each core handles 32 (atomic,
batch) rows x all N entities.

Symbolic message passing runs as a GpSimd local_scatter permutation network:
  hv (src-partition layout) --local_scatter--> staging --PE transpose-->
  B (dst-partition layout) --local_scatter--> M (per-edge, dst-ordered)
  --x val--> --local_scatter--> dense sym3 [128, T, R]
with duplicate-src edges resolved by sibling-read passes over M and
duplicate-dst edges by pair slots + spill layers.

Neural side (score matmul, softmax, clip-norm chain, projection) is a dense
PE/DVE pipeline over entity tiles streamed from HBM in bf16.
"""

import numpy as np
import ml_dtypes

A, B, N, D, E = 2, 128, 50000, 512, 4096
CLIP = 1e-14

NCORES = 8
BLOC = B // NCORES          # 16 batch per core
R = A * BLOC                # 32 rows per core
P = 128
T = 391                     # n tiles: 391*128 = 50048
NPAD = T * P                # 50048
NWIN = 13                   # score psum windows of 32 tiles
TPAD = NWIN * 32            # 416

# symbolic permutation network constants
NG = 4                      # row groups for staging
GR = R // NG                # 8 rows per group
C1 = 13                     # staging chunks per group
TCH = 56                    # t-tiles per M chunk
NCH = 7                     # M chunks
SING = 224                  # singles per (p, chunk)
PAIR = 16                   # pair slots per (p, chunk)
MSTRIDE = SING + 2 * PAIR   # 256
C2 = 6                      # dupe staging chunks
M2CAP = 24                  # M2 capacity per (p, chunk)
NDUPE = 3                   # dupe passes
SPSTRIDE = PAIR + NDUPE * M2CAP  # 88
NSPILL = 3                  # spill layers
SYM_LEN = T * R             # 12512
BLEN = NG * C1 * P          # 6656

bf16 = ml_dtypes.bfloat16

# spill build plan: set of (layer, chunk) with any entry on any core.
# populated by _build_host_inputs before build_program is called.
_SPILL_PLAN = [(L, c) for L in range(NSPILL) for c in range(NCH)]


def _chunk_elems(ch):
    t0, t1 = ch * TCH, min((ch + 1) * TCH, T)
    return (t1 - t0) * R


def _cumcount(keys, sel):
    """For each selected element, its 0-based rank among same-key elements."""
    idxs = np.nonzero(sel)[0]
    if idxs.size == 0:
        return idxs, np.zeros(0, np.int64)
    kk = keys[idxs]
    o = np.argsort(kk, kind="stable")
    k = kk[o]
    n = np.ones(idxs.size, bool)
    n[1:] = k[1:] != k[:-1]
    g = np.cumsum(n) - 1
    f = np.zeros(g.max() + 1, np.int64)
    f[g[n]] = np.nonzero(n)[0]
    cnt_sorted = np.arange(idxs.size) - f[g]
    out = np.empty(idxs.size, np.int64)
    out[o] = cnt_sorted
    return idxs, out


def _prep_symbolic(hv_core, src, dst, val):
    """hv_core [R, NPAD] f32, src/dst/val [R, E]. Returns device arrays +
    spill plan entries."""
    r = np.repeat(np.arange(R), E)
    s = src.ravel().astype(np.int64)
    d = dst.ravel().astype(np.int64)
    v = val.ravel().astype(np.float32)
    NE = r.size

    p_s, t_s = s % P, s // P
    p_d, t_d = d % P, d // P
    g = r // GR
    ch = t_d // TCH

    # src-group ranks
    key_s = r * NPAD + s
    o = np.argsort(key_s, kind="stable")
    ks = key_s[o]
    newg = np.ones(NE, bool)
    newg[1:] = ks[1:] != ks[:-1]
    gid = np.cumsum(newg) - 1
    first = np.zeros(gid.max() + 1, np.int64)
    first[gid[newg]] = np.nonzero(newg)[0]
    srank = np.empty(NE, np.int64)
    srank[o] = np.arange(NE) - first[gid]
    assert srank.max() <= NDUPE, f"src dupe depth {srank.max()}"
    main = srank == 0
    sibling = np.empty(NE, np.int64)
    sibling[o] = o[first[gid]]

    # staging chunks for main edges
    mi, c1 = _cumcount((g * P + p_s) * P + p_d, main)
    assert c1.max() < C1, f"C1 overflow {c1.max() + 1}"

    # dst groups: mains first
    key_d = r * NPAD + d
    od = np.argsort(key_d * 8 + np.minimum(srank, 7), kind="stable")
    kd = key_d[od]
    ngd = np.ones(NE, bool)
    ngd[1:] = kd[1:] != kd[:-1]
    gidd = np.cumsum(ngd) - 1
    fd = np.zeros(gidd.max() + 1, np.int64)
    fd[gidd[ngd]] = np.nonzero(ngd)[0]
    drank = np.empty(NE, np.int64)
    drank[od] = np.arange(NE) - fd[gidd]
    gmain = np.zeros(gidd.max() + 1, np.int64)
    np.add.at(gmain, gidd, main[od].astype(np.int64))
    qmain = np.empty(NE, np.int64)
    qmain[od] = gmain[gidd]

    is_single = main & (qmain == 1)
    is_pairm = main & (qmain >= 2)
    cellM = p_d * NCH + ch

    si, scnt = _cumcount(cellM, is_single)
    assert scnt.size == 0 or scnt.max() < SING, f"SING overflow {scnt.max()+1}"
    pair_lead = is_pairm & (drank % 2 == 0)
    pi, pcnt = _cumcount(cellM, pair_lead)
    assert pcnt.size == 0 or pcnt.max() < PAIR, f"PAIR overflow {pcnt.max()+1}"
    pairslot = {(key_d[e], drank[e] // 2): c for e, c in zip(pi, pcnt)}

    Mslot = np.full(NE, -1, np.int64)
    Mslot[si] = ch[si] * MSTRIDE + scnt
    pm = np.nonzero(is_pairm)[0]
    for e in pm:
        c = pairslot[(key_d[e], drank[e] // 2)]
        Mslot[e] = ch[e] * MSTRIDE + SING + 2 * c + (drank[e] % 2)

    m2slot = np.full(NE, -1, np.int64)
    m2pass = np.full(NE, -1, np.int64)
    for k in range(NDUPE):
        ki, kcnt = _cumcount(cellM, srank == k + 1)
        assert kcnt.size == 0 or kcnt.max() < M2CAP, f"M2 overflow {kcnt.max()+1}"
        m2slot[ki] = ch[ki] * M2CAP + kcnt
        m2pass[ki] = k

    c2 = np.full(NE, -1, np.int64)
    for k in range(NDUPE):
        ki, cko = _cumcount(p_d[sibling] * P + p_d, srank == k + 1)
        assert cko.size == 0 or cko.max() < C2, f"C2 overflow {cko.max()+1}"
        c2[ki] = cko

    # spill layers
    sym_of = t_d * R + r
    spill_ctr = {}
    pair_spill = {}
    for e in pi:
        kk = (p_d[e], sym_of[e])
        L = spill_ctr.get(kk, 0)
        assert L < NSPILL, "spill overflow"
        spill_ctr[kk] = L + 1
        pair_spill[(key_d[e], drank[e] // 2)] = L
    m2_spill = np.full(NE, -1, np.int64)
    for e in np.nonzero(srank >= 1)[0]:
        kk = (p_d[e], sym_of[e])
        L = spill_ctr.get(kk, 0)
        assert L < NSPILL, "spill overflow"
        spill_ctr[kk] = L + 1
        m2_spill[e] = L

    # ---- device arrays
    arr = {}
    idx1 = np.full((NG, P, GR * T), -1, np.int16)
    ma = np.nonzero(main)[0]
    idx1[g[ma], p_s[ma], (r[ma] % GR) * T + t_s[ma]] = \
        (c1 * P + p_d[mi]).astype(np.int16)
    arr["idx1"] = idx1

    idx2 = np.full((P, BLEN), -1, np.int16)
    idx2[p_d[mi], (g[mi] * C1 + c1) * P + p_s[mi]] = Mslot[mi].astype(np.int16)
    arr["idx2"] = idx2

    valM = np.zeros((P, NCH * MSTRIDE), np.float32)
    valM[p_d[mi], Mslot[mi]] = v[mi]
    arr["valM"] = valM.astype(bf16)

    idxd = np.full((NDUPE, P, NCH * MSTRIDE), -1, np.int16)
    idx2d = np.full((NDUPE, P, C2 * P), -1, np.int16)
    valM2 = np.zeros((NDUPE, P, NCH * M2CAP), np.float32)
    for k in range(NDUPE):
        ki = np.nonzero(srank == k + 1)[0]
        sib = sibling[ki]
        idxd[k, p_d[sib], Mslot[sib]] = (c2[ki] * P + p_d[ki]).astype(np.int16)
        idx2d[k, p_d[ki], c2[ki] * P + p_d[sib]] = m2slot[ki].astype(np.int16)
        valM2[k, p_d[ki], m2slot[ki]] = v[ki]
    arr["idxd"] = idxd
    arr["idx2d"] = idx2d
    arr["valM2"] = valM2.astype(bf16)

    idx3 = np.full((P, NCH * SING), -1, np.int16)
    idx3[p_d[si], ch[si] * SING + scnt] = \
        ((t_d[si] - ch[si] * TCH) * R + r[si]).astype(np.int16)
    arr["idx3"] = idx3

    idxsp = np.full((NSPILL, P, NCH * SPSTRIDE), -1, np.int16)
    plan = set()
    for e in pi:
        c = pairslot[(key_d[e], drank[e] // 2)]
        L = pair_spill[(key_d[e], drank[e] // 2)]
        idxsp[L, p_d[e], ch[e] * SPSTRIDE + c] = \
            (t_d[e] - ch[e] * TCH) * R + r[e]
        plan.add((L, int(ch[e])))
    for e in np.nonzero(srank >= 1)[0]:
        k = m2pass[e]
        L = m2_spill[e]
        rel = m2slot[e] - ch[e] * M2CAP
        idxsp[L, p_d[e], ch[e] * SPSTRIDE + PAIR + k * M2CAP + rel] = \
            (t_d[e] - ch[e] * TCH) * R + r[e]
        plan.add((int(L), int(ch[e])))
    arr["idxsp"] = idxsp

    arr["hv_sb"] = np.ascontiguousarray(
        hv_core.reshape(R, T, P).transpose(2, 0, 1).reshape(P, R * T)
    ).astype(bf16)
    return arr, plan


def _prep_core(core, head_vector, head_emb, pred_emb, edge_val, edge_src,
               edge_dst):
    b0 = core * BLOC
    hv = np.zeros((R, NPAD), np.float32)
    ev = np.empty((R, E), np.float32)
    es = np.empty((R, E), np.int64)
    ed = np.empty((R, E), np.int64)
    hemb = np.empty((R, D), np.float32)
    pemb = np.empty((R, D), np.float32)
    for a in range(A):
        sl = slice(a * BLOC, (a + 1) * BLOC)
        hv[sl, :N] = head_vector[a, b0:b0 + BLOC]
        ev[sl] = edge_val[a, b0:b0 + BLOC]
        es[sl] = edge_src[a, b0:b0 + BLOC]
        ed[sl] = edge_dst[a, b0:b0 + BLOC]
        hemb[sl] = head_emb[a, b0:b0 + BLOC]
        pemb[sl] = pred_emb[a, b0:b0 + BLOC]

    arr, plan = _prep_symbolic(hv, es, ed, ev)
    arr["hemb"] = hemb
    arr["pemb"] = pemb
    return arr, plan


def _build_host_inputs(entity_embedding, head_vector, head_emb, pred_emb,
                       edge_val, edge_src, edge_dst):
    global _SPILL_PLAN
    entity_pad = np.zeros((NPAD, D), np.float32)
    entity_pad[:N] = entity_embedding
    # proj moving operand entN: [g, p, k*512+dd] = entity[(8g+k)*128+p, dd]
    npadg = ((T + 7) // 8) * 8 * P
    en_t = np.zeros((npadg // P, P, D), np.float32)
    en_t[:T] = entity_pad.reshape(T, P, D)
    entN = np.ascontiguousarray(
        en_t.reshape(-1, 8, P, D).transpose(0, 2, 1, 3).reshape(-1, P, 8 * D)
    ).astype(bf16)
    # score stationary entT: [g, p, k*512+dk*128+j] = entity[(13g+k)*128+j, dk*128+p]
    ngrp = (T + NWIN - 1) // NWIN
    et_t = np.zeros((ngrp * NWIN, P, D), np.float32)
    et_t[:T] = entity_pad.reshape(T, P, D)
    et5 = et_t.reshape(ngrp, NWIN, P, 4, P).transpose(0, 4, 1, 3, 2)
    entT = np.ascontiguousarray(et5.reshape(ngrp, P, NWIN * D)).astype(bf16)
    ident = np.eye(P, dtype=np.float32)
    identb = np.eye(P, dtype=bf16)
    ones128 = np.ones((P, 1), np.float32)
    padm = np.zeros((P, 1), np.float32)
    padm[N - (T - 1) * P:] = -1e30

    in_maps = []
    plan_all = set()
    for core in range(NCORES):
        m, plan = _prep_core(core, head_vector, head_emb, pred_emb,
                             edge_val, edge_src, edge_dst)
        plan_all |= plan
        m["entT"] = entT
        m["entN"] = entN
        m["ident"] = ident
        m["identb"] = identb
        m["ones128"] = ones128
        m["padm"] = padm
        in_maps.append(m)
    _SPILL_PLAN = sorted(plan_all)
    return in_maps


# ---------------------------------------------------------------------------
# Bass program
# ---------------------------------------------------------------------------

def build_program():
    from contextlib import ExitStack
    import concourse.bass as bass
    import concourse.tile as tile
    from concourse import bacc, mybir
    from concourse.alu_op_type import AluOpType as op
    import bass_rust

    dt = mybir.dt
    f32, b16, i16 = dt.float32, dt.bfloat16, dt.int16
    Exp = bass_rust.ActivationFunctionType.Exp

    nc = bacc.Bacc("TRN2", target_bir_lowering=False, debug=False,
                   num_devices=NCORES)

    def din(name, shape, dtype):
        return nc.dram_tensor(name, list(shape), dtype, kind="ExternalInput").ap()

    NGRP = (T + NWIN - 1) // NWIN
    NPGRP = (T + 7) // 8
    entT = din("entT", (NGRP, P, NWIN * D), b16)
    entN = din("entN", (NPGRP, P, 8 * D), b16)
    hemb = din("hemb", (R, D), f32)
    pemb = din("pemb", (R, D), f32)
    ident = din("ident", (P, P), f32)
    identb = din("identb", (P, P), b16)
    ones128 = din("ones128", (P, 1), f32)
    padm = din("padm", (P, 1), f32)
    hv_sb_d = din("hv_sb", (P, R * T), b16)
    idx1_d = din("idx1", (NG, P, GR * T), i16)
    idx2_d = din("idx2", (P, BLEN), i16)
    valM_d = din("valM", (P, NCH * MSTRIDE), b16)
    idxd_d = din("idxd", (NDUPE, P, NCH * MSTRIDE), i16)
    idx2d_d = din("idx2d", (NDUPE, P, C2 * P), i16)
    valM2_d = din("valM2", (NDUPE, P, NCH * M2CAP), b16)
    idx3_d = din("idx3", (P, NCH * SING), i16)
    idxsp_d = din("idxsp", (NSPILL, P, NCH * SPSTRIDE), i16)

    out = nc.dram_tensor("out", [BLOC, D], f32, kind="ExternalOutput").ap()

    with tile.TileContext(nc) as tc, ExitStack() as ctx:
        ctx.enter_context(nc.allow_low_precision(
            reason="bf16 storage is deliberate; reductions accumulate f32"))
        const = ctx.enter_context(tc.tile_pool(name="const", bufs=1))
        big = ctx.enter_context(tc.tile_pool(name="big", bufs=1))
        small = ctx.enter_context(tc.tile_pool(name="small", bufs=1))
        sym = ctx.enter_context(tc.tile_pool(name="sym", bufs=1))
        symi = ctx.enter_context(tc.tile_pool(name="symi", bufs=2))
        entp = ctx.enter_context(tc.tile_pool(name="entp", bufs=2))
        ps_score = ctx.enter_context(tc.tile_pool(name="ps_score", bufs=2, space="PSUM"))
        ps_tr = ctx.enter_context(tc.tile_pool(name="ps_tr", bufs=1, space="PSUM"))
        ps_trb = ctx.enter_context(tc.tile_pool(name="ps_trb", bufs=2, space="PSUM"))
        ps_proj = ctx.enter_context(tc.tile_pool(name="ps_proj", bufs=1, space="PSUM"))
        early = ExitStack()
        early1 = early.enter_context(tc.tile_pool(name="early1", bufs=2))
        earlys = early.enter_context(tc.tile_pool(name="earlys", bufs=4))
        earlyb = early.enter_context(tc.tile_pool(name="earlyb", bufs=1))
        enpool = []   # created after early pools close

        # ---- constants
        ident_sb = const.tile([P, P], f32)
        nc.sync.dma_start(ident_sb[:], ident[:])
        identb_sb = const.tile([P, P], b16)
        nc.sync.dma_start(identb_sb[:], identb[:])
        ones128_sb = const.tile([P, 1], f32)
        nc.sync.dma_start(ones128_sb[:], ones128[:])
        padm_sb = const.tile([P, 1], f32)
        nc.sync.dma_start(padm_sb[:], padm[:])

        # ---- symbolic small input DMAs (scalar-engine DMA queue)
        idx2_t = earlyb.tile([P, BLEN], i16, tag="idx2")
        nc.scalar.dma_start(idx2_t[:], idx2_d[:])
        valM_t = sym.tile([P, NCH * MSTRIDE], b16, tag="valM")
        nc.scalar.dma_start(valM_t[:], valM_d[:])
        idx2d_t = sym.tile([P, NDUPE, C2 * P], i16, tag="idx2d")
        nc.scalar.dma_start(idx2d_t[:], idx2d_d[:].rearrange("k p n -> p k n"))
        valM2_t = sym.tile([P, NDUPE, NCH * M2CAP], b16, tag="valM2")
        nc.scalar.dma_start(valM2_t[:], valM2_d[:].rearrange("k p n -> p k n"))
        idx3_t = sym.tile([P, NCH * SING], i16, tag="idx3")
        nc.scalar.dma_start(idx3_t[:], idx3_d[:])
        idxsp_t = sym.tile([P, NSPILL, NCH * SPSTRIDE], i16, tag="idxsp")
        nc.scalar.dma_start(idxsp_t[:], idxsp_d[:].rearrange("k p n -> p k n"))

        # ---- tail = head + pred -> tailT [128d, dk*32+r] bf16
        tail_f = small.tile([R, D], f32, tag="tailf")
        h_t = small.tile([R, D], f32, tag="hp")
        p_t = small.tile([R, D], f32, tag="hp2")
        nc.sync.dma_start(h_t[:], hemb[:])
        nc.sync.dma_start(p_t[:], pemb[:])
        nc.vector.tensor_tensor(out=tail_f[:], in0=h_t[:], in1=p_t[:], op=op.add)
        tailT = const.tile([P, 4 * R], b16)
        for dk in range(4):
            pt = ps_tr.tile([P, 512], f32, tag="tr")
            nc.tensor.transpose(out=pt[:, :R], in_=tail_f[:, dk * P:(dk + 1) * P],
                                identity=ident_sb[:R, :R])
            nc.vector.tensor_copy(out=tailT[:, dk * R:(dk + 1) * R], in_=pt[:, :R])

        # ---- symbolic stage 1: 4 x (dma hv slice + local_scatter)
        stg = []
        for g in range(NG):
            hvg = early1.tile([P, GR * T], b16, tag="hvg")
            nc.scalar.dma_start(hvg[:], hv_sb_d[:, g * GR * T:(g + 1) * GR * T])
            i1 = early1.tile([P, GR * T], i16, tag="idx1")
            nc.scalar.dma_start(i1[:], idx1_d[g])
            st = earlys.tile([P, C1 * P], b16, tag="stg")
            nc.gpsimd.local_scatter(st[:], hvg[:], i1[:], channels=P,
                                    num_elems=C1 * P, num_idxs=GR * T)
            stg.append(st)

        Bt = earlyb.tile([P, BLEN], b16, tag="B")
        M_t = sym.tile([P, NCH * MSTRIDE], b16, tag="M")
        M2 = []
        spill = sym.tile([P, NCH, SPSTRIDE], b16, tag="spill")
        sym3 = big.tile([P, T, R], b16, tag="bigB")
        sym3f = sym3[:].rearrange("p t r -> p (t r)")

        def do_b_transposes():
            for g in range(NG):
                for c in range(C1):
                    pt = ps_trb.tile([P, 512], b16, tag="trb")
                    nc.tensor.transpose(out=pt[:, :P],
                                        in_=stg[g][:, c * P:(c + 1) * P],
                                        identity=identb_sb[:])
                    nc.vector.tensor_copy(
                        out=Bt[:, (g * C1 + c) * P:(g * C1 + c + 1) * P],
                        in_=pt[:, :P])
            nc.gpsimd.local_scatter(M_t[:], Bt[:], idx2_t[:], channels=P,
                                    num_elems=NCH * MSTRIDE, num_idxs=BLEN)

        def do_dupe_pass(k):
            idxd_t = symi.tile([P, NCH * MSTRIDE], i16, tag="idxd")
            nc.scalar.dma_start(idxd_t[:], idxd_d[k])
            st2 = symi.tile([P, C2 * P], b16, tag="stg2")
            nc.gpsimd.local_scatter(st2[:], M_t[:], idxd_t[:], channels=P,
                                    num_elems=C2 * P, num_idxs=NCH * MSTRIDE)
            Bd = symi.tile([P, C2 * P], b16, tag="Bd")
            for c in range(C2):
                pt = ps_trb.tile([P, 512], b16, tag="trb")
                nc.tensor.transpose(out=pt[:, :P], in_=st2[:, c * P:(c + 1) * P],
                                    identity=identb_sb[:])
                nc.vector.tensor_copy(out=Bd[:, c * P:(c + 1) * P], in_=pt[:, :P])
            m2t = sym.tile([P, NCH * M2CAP], b16, tag=f"M2_{k}")
            nc.gpsimd.local_scatter(m2t[:], Bd[:], idx2d_t[:, k, :], channels=P,
                                    num_elems=NCH * M2CAP, num_idxs=C2 * P)
            M2.append(m2t)

        def close_early():
            early.close()
            enp = ctx.enter_context(tc.tile_pool(name="enp", bufs=3))
            enpool.append(enp)

        def do_symbolic_finish():
            nc.vector.tensor_tensor(out=M_t[:], in0=M_t[:], in1=valM_t[:],
                                    op=op.mult)
            a = M_t[:]
            in0 = bass.AP(a.tensor, a.offset + SING,
                          [list(a.ap[0]), [MSTRIDE, NCH], [2, PAIR]])
            in1 = bass.AP(a.tensor, a.offset + SING + 1,
                          [list(a.ap[0]), [MSTRIDE, NCH], [2, PAIR]])
            nc.vector.tensor_tensor(out=spill[:, :, :PAIR], in0=in0, in1=in1,
                                    op=op.add)
            for k in range(NDUPE):
                m2v = M2[k][:].rearrange("p (ch s) -> p ch s", s=M2CAP)
                nc.vector.tensor_tensor(
                    out=m2v, in0=m2v,
                    in1=valM2_t[:, k, :].rearrange("p (ch s) -> p ch s", s=M2CAP),
                    op=op.mult)
                nc.vector.tensor_copy(
                    out=spill[:, :, PAIR + k * M2CAP:PAIR + (k + 1) * M2CAP],
                    in_=m2v)
            for c in range(NCH):
                ne = _chunk_elems(c)
                nc.gpsimd.local_scatter(
                    sym3f[:, c * TCH * R: c * TCH * R + ne],
                    M_t[:, c * MSTRIDE:c * MSTRIDE + SING],
                    idx3_t[:, c * SING:(c + 1) * SING],
                    channels=P, num_elems=ne, num_idxs=SING)
            for (L, c) in _SPILL_PLAN:
                ne = _chunk_elems(c)
                ssp = sym.tile([P, TCH * R], b16, tag="ssp")
                nc.gpsimd.local_scatter(
                    ssp[:, :ne], spill[:, c, :],
                    idxsp_t[:, L, c * SPSTRIDE:(c + 1) * SPSTRIDE],
                    channels=P, num_elems=ne, num_idxs=SPSTRIDE)
                nc.vector.tensor_tensor(
                    out=sym3f[:, c * TCH * R: c * TCH * R + ne],
                    in0=sym3f[:, c * TCH * R: c * TCH * R + ne],
                    in1=ssp[:, :ne], op=op.add)

        # ---- helpers
        def col_reduce_bcast(x_rc, nr, op_red, post, tagp):
            pt = ps_tr.tile([P, 512], f32, tag="tr")
            nc.tensor.transpose(out=pt[:nr, :P], in_=x_rc, identity=ident_sb[:])
            v = small.tile([nr, 1], f32, tag="v" + tagp)
            nc.vector.tensor_reduce(out=v[:], in_=pt[:nr, :P],
                                    axis=mybir.AxisListType.X, op=op_red)
            v2 = small.tile([nr, 1], b16, tag="w" + tagp)
            post(v, v2)
            pb = ps_trb.tile([P, 512], b16, tag="trb")
            vb = v2[:].to_broadcast([nr, P])
            nc.tensor.transpose(out=pb[:, :nr], in_=vb, identity=identb_sb[:nr, :nr])
            return pb

        def bcast8(ps_tile, r0, cnt8, nr):
            a = ps_tile[:, r0:r0 + nr]
            return bass.AP(a.tensor, a.offset,
                           [list(a.ap[0]), [0, cnt8], list(a.ap[1])])

        sy_flat = sym3[:].rearrange("p t r -> p (t r)")
        sy_rt = sym3[:].rearrange("p t r -> p r t")

        y1 = small.tile([P, R], f32, tag="y1")

        def do_y1():
            nc.vector.tensor_reduce(out=y1[:], in_=sy_rt,
                                    axis=mybir.AxisListType.X, op=op.add)

        hooks = {10: do_b_transposes,
                 12: close_early,
                 16: lambda: do_dupe_pass(0),
                 17: lambda: do_dupe_pass(1),
                 18: lambda: do_dupe_pass(2),
                 20: do_symbolic_finish,
                 27: do_y1}

        # ---- score: per group: matmuls -> copy -> (mask) -> exp -> s1 partial
        score3 = big.tile([P, T, R], b16, tag="bigA")
        s1p = small.tile([P, NGRP, R], f32, tag="s1p")
        for grp in range(NGRP):
            if grp in hooks:
                hooks[grp]()
            t0 = grp * NWIN
            cnt = min(NWIN, T - t0)
            et = entp.tile([P, NWIN, 4, P], b16, tag="et")
            nc.sync.dma_start(et[:], entT[grp])
            pss = ps_score.tile([P, TPAD], f32, tag="ps_s")
            for k in range(cnt):
                for dk in range(4):
                    nc.tensor.matmul(out=pss[:, 32 * k:32 * k + 32],
                                     lhsT=et[:, k, dk, :],
                                     rhs=tailT[:, dk * R:(dk + 1) * R],
                                     start=(dk == 0), stop=(dk == 3))
            nc.vector.tensor_copy(out=score3[:, t0:t0 + cnt, :],
                                  in_=pss[:, :32 * cnt])
            if grp == NGRP - 1:
                nc.vector.tensor_scalar(out=score3[:, T - 1:T, :],
                                        in0=score3[:, T - 1:T, :],
                                        scalar1=padm_sb[:], scalar2=None,
                                        op0=op.add)
            sl_flat = score3[:, t0:t0 + cnt, :].rearrange("p t r -> p (t r)")
            nc.scalar.activation(out=sl_flat, in_=sl_flat, func=Exp)
            nc.vector.tensor_reduce(
                out=s1p[:, grp, :],
                in_=score3[:, t0:t0 + cnt, :].rearrange("p t r -> p r t"),
                axis=mybir.AxisListType.X, op=op.add)

        # ---- normalizers: sB = 1/sum(exp), yB = 1/max(CLIP, sum(sym))
        s1 = small.tile([P, R], f32, tag="s1")
        nc.vector.tensor_reduce(out=s1[:], in_=s1p[:].rearrange("p g r -> p r g"),
                                axis=mybir.AxisListType.X, op=op.add)

        def recip_post(v, v2):
            nc.vector.reciprocal(out=v2[:], in_=v[:])

        def clipmax_recip(v, v2):
            nc.vector.tensor_scalar(out=v[:], in0=v[:], scalar1=float(CLIP),
                                    scalar2=None, op0=op.max)
            nc.vector.reciprocal(out=v2[:], in_=v[:])

        sB = col_reduce_bcast(s1[:], R, op.add, recip_post, "s")
        yB = col_reduce_bcast(y1[:], R, op.add, clipmax_recip, "y")

        # ---- fused agg + projection loop over 8-tile groups
        # agg_unnorm = (sym0*y0 + e0*s0) * (sym1*y1 + e1*s1); norm deferred
        agg_b = big.tile([P, T, BLOC], b16, tag="aggb")
        pso = ps_proj.tile([BLOC, D], f32)
        ft1 = small.tile([P, 8, BLOC], f32, tag="ft1")
        ft2 = small.tile([P, 8, BLOC], f32, tag="ft2")
        ft3 = small.tile([P, 8, BLOC], f32, tag="ft3")
        enp = enpool[0]
        for g in range(NPGRP):
            nt0 = 8 * g
            cnt8 = min(8, T - nt0)
            en = enp.tile([P, 8, D], b16, tag="en")
            nc.sync.dma_start(en[:], entN[g])
            sl = slice(nt0, nt0 + cnt8)
            nc.vector.tensor_tensor(out=ft1[:, :cnt8], in0=sym3[:, sl, :BLOC],
                                    in1=bcast8(yB, 0, cnt8, BLOC), op=op.mult)
            nc.vector.tensor_tensor(out=ft2[:, :cnt8], in0=score3[:, sl, :BLOC],
                                    in1=bcast8(sB, 0, cnt8, BLOC), op=op.mult)
            nc.vector.tensor_tensor(out=ft1[:, :cnt8], in0=ft1[:, :cnt8],
                                    in1=ft2[:, :cnt8], op=op.add)
            nc.vector.tensor_tensor(out=ft2[:, :cnt8], in0=sym3[:, sl, BLOC:],
                                    in1=bcast8(yB, BLOC, cnt8, BLOC), op=op.mult)
            nc.vector.tensor_tensor(out=ft3[:, :cnt8], in0=score3[:, sl, BLOC:],
                                    in1=bcast8(sB, BLOC, cnt8, BLOC), op=op.mult)
            nc.vector.tensor_tensor(out=ft2[:, :cnt8], in0=ft2[:, :cnt8],
                                    in1=ft3[:, :cnt8], op=op.add)
            nc.vector.tensor_tensor(out=agg_b[:, sl, :], in0=ft1[:, :cnt8],
                                    in1=ft2[:, :cnt8], op=op.mult)
            for k in range(cnt8):
                nt = nt0 + k
                nc.tensor.matmul(out=pso[:], lhsT=agg_b[:, nt, :],
                                 rhs=en[:, k, :],
                                 start=(nt == 0), stop=(nt == T - 1))

        # ---- denominator: sum over n of agg_unnorm, then out = pso / denom
        g1 = small.tile([P, BLOC], f32, tag="g1")
        nc.vector.tensor_reduce(out=g1[:],
                                in_=agg_b[:].rearrange("p t r -> p r t"),
                                axis=mybir.AxisListType.X, op=op.add)
        pd = ps_tr.tile([P, 512], f32, tag="tr")
        nc.tensor.matmul(out=pd[:BLOC, :1], lhsT=g1[:], rhs=ones128_sb[:],
                         start=True, stop=True)
        dclip = small.tile([BLOC, 1], f32, tag="dclip")
        nc.vector.tensor_scalar(out=dclip[:], in0=pd[:BLOC, :1],
                                scalar1=float(CLIP), scalar2=None, op0=op.max)
        drcp = small.tile([BLOC, 1], f32, tag="drcp")
        nc.vector.reciprocal(out=drcp[:], in_=dclip[:])
        out_sb = small.tile([BLOC, D], f32, tag="outsb")
        nc.vector.tensor_scalar(out=out_sb[:], in0=pso[:], scalar1=drcp[:],
                                scalar2=None, op0=op.mult)
        nc.sync.dma_start(out[:], out_sb[:])

    nc.compile()
    return nc


_PROGRAM = None


def kernel(entity_embedding, head_vector, head_emb, pred_emb,
           edge_val, edge_src, edge_dst):
    global _PROGRAM
    from concourse.bass_utils import run_bass_kernel_spmd

    in_maps = _build_host_inputs(entity_embedding, head_vector,
                                 head_emb, pred_emb,
                                 edge_val, edge_src, edge_dst)
    if _PROGRAM is None:
        _PROGRAM = build_program()
    res = run_bass_kernel_spmd(_PROGRAM, in_maps, list(range(NCORES)))
    out = np.empty((B, D), np.float32)
    for c in range(NCORES):
        out[c * BLOC:(c + 1) * BLOC] = res.results[c]["out"]
    return out


if __name__ == "__main__":
    import reference
    inputs = {k: np.asarray(v) for k, v in reference.setup_inputs().items()}
    got = kernel(**inputs)
    want = np.asarray(reference.reference(**inputs))
    err = np.abs(got - want).max() / np.abs(want).max()
    print("Relative error:", err)


# revision 7
# speedup vs baseline: 3.7789x; 1.0620x over previous
"""Trainium2 Bass kernel for NeuralSymbolicMP layer (gnn_message_passing).

Batch-sharded over B across 8 NeuronCores; each core handles 32 (atomic,
batch) rows x all N entities.

Symbolic message passing runs as a GpSimd local_scatter permutation network:
  hv (src-partition layout) --local_scatter--> staging --PE transpose-->
  B (dst-partition layout) --local_scatter--> M (per-edge, dst-ordered)
  --x val--> --local_scatter--> dense sym3 [128, T, R]
with duplicate-src edges resolved by sibling-read passes over M and
duplicate-dst edges by pair slots + spill layers.

Neural side (score matmul, softmax, clip-norm chain, projection) is a dense
PE/DVE pipeline over entity tiles streamed from HBM in bf16.
"""

import numpy as np
import ml_dtypes

A, B, N, D, E = 2, 128, 50000, 512, 4096
CLIP = 1e-14

NCORES = 8
BLOC = B // NCORES          # 16 batch per core
R = A * BLOC                # 32 rows per core
P = 128
T = 391                     # n tiles: 391*128 = 50048
NPAD = T * P                # 50048
NWIN = 13                   # score psum windows of 32 tiles
TPAD = NWIN * 32            # 416

# symbolic permutation network constants
NG = 4                      # row groups for staging
GR = R // NG                # 8 rows per group
C1 = 13                     # staging chunks per group
TCH = 56                    # t-tiles per M chunk
NCH = 7                     # M chunks
SING = 224                  # singles per (p, chunk)
PAIR = 16                   # pair slots per (p, chunk)
MSTRIDE = SING + 2 * PAIR   # 256
C2 = 6                      # dupe staging chunks
M2CAP = 24                  # M2 capacity per (p, chunk)
NDUPE = 3                   # dupe passes
SPSTRIDE = PAIR + NDUPE * M2CAP  # 88
NSPILL = 3                  # spill layers
SYM_LEN = T * R             # 12512
BLEN = NG * C1 * P          # 6656

bf16 = ml_dtypes.bfloat16

# spill build plan: set of (layer, chunk) with any entry on any core.
# populated by _build_host_inputs before build_program is called.
_SPILL_PLAN = [(L, c) for L in range(NSPILL) for c in range(NCH)]


def _chunk_elems(ch):
    t0, t1 = ch * TCH, min((ch + 1) * TCH, T)
    return (t1 - t0) * R


def _cumcount(keys, sel):
    """For each selected element, its 0-based rank among same-key elements."""
    idxs = np.nonzero(sel)[0]
    if idxs.size == 0:
        return idxs, np.zeros(0, np.int64)
    kk = keys[idxs]
    o = np.argsort(kk, kind="stable")
    k = kk[o]
    n = np.ones(idxs.size, bool)
    n[1:] = k[1:] != k[:-1]
    g = np.cumsum(n) - 1
    f = np.zeros(g.max() + 1, np.int64)
    f[g[n]] = np.nonzero(n)[0]
    cnt_sorted = np.arange(idxs.size) - f[g]
    out = np.empty(idxs.size, np.int64)
    out[o] = cnt_sorted
    return idxs, out


def _prep_symbolic(hv_core, src, dst, val):
    """hv_core [R, NPAD] f32, src/dst/val [R, E]. Returns device arrays +
    spill plan entries."""
    r = np.repeat(np.arange(R), E)
    s = src.ravel().astype(np.int64)
    d = dst.ravel().astype(np.int64)
    v = val.ravel().astype(np.float32)
    NE = r.size

    p_s, t_s = s % P, s // P
    p_d, t_d = d % P, d // P
    g = r // GR
    ch = t_d // TCH

    # src-group ranks
    key_s = r * NPAD + s
    o = np.argsort(key_s, kind="stable")
    ks = key_s[o]
    newg = np.ones(NE, bool)
    newg[1:] = ks[1:] != ks[:-1]
    gid = np.cumsum(newg) - 1
    first = np.zeros(gid.max() + 1, np.int64)
    first[gid[newg]] = np.nonzero(newg)[0]
    srank = np.empty(NE, np.int64)
    srank[o] = np.arange(NE) - first[gid]
    assert srank.max() <= NDUPE, f"src dupe depth {srank.max()}"
    main = srank == 0
    sibling = np.empty(NE, np.int64)
    sibling[o] = o[first[gid]]

    # staging chunks for main edges
    mi, c1 = _cumcount((g * P + p_s) * P + p_d, main)
    assert c1.max() < C1, f"C1 overflow {c1.max() + 1}"

    # dst groups: mains first
    key_d = r * NPAD + d
    od = np.argsort(key_d * 8 + np.minimum(srank, 7), kind="stable")
    kd = key_d[od]
    ngd = np.ones(NE, bool)
    ngd[1:] = kd[1:] != kd[:-1]
    gidd = np.cumsum(ngd) - 1
    fd = np.zeros(gidd.max() + 1, np.int64)
    fd[gidd[ngd]] = np.nonzero(ngd)[0]
    drank = np.empty(NE, np.int64)
    drank[od] = np.arange(NE) - fd[gidd]
    gmain = np.zeros(gidd.max() + 1, np.int64)
    np.add.at(gmain, gidd, main[od].astype(np.int64))
    qmain = np.empty(NE, np.int64)
    qmain[od] = gmain[gidd]

    is_single = main & (qmain == 1)
    is_pairm = main & (qmain >= 2)
    cellM = p_d * NCH + ch

    si, scnt = _cumcount(cellM, is_single)
    assert scnt.size == 0 or scnt.max() < SING, f"SING overflow {scnt.max()+1}"
    pair_lead = is_pairm & (drank % 2 == 0)
    pi, pcnt = _cumcount(cellM, pair_lead)
    assert pcnt.size == 0 or pcnt.max() < PAIR, f"PAIR overflow {pcnt.max()+1}"
    pairslot = {(key_d[e], drank[e] // 2): c for e, c in zip(pi, pcnt)}

    Mslot = np.full(NE, -1, np.int64)
    Mslot[si] = ch[si] * MSTRIDE + scnt
    pm = np.nonzero(is_pairm)[0]
    for e in pm:
        c = pairslot[(key_d[e], drank[e] // 2)]
        Mslot[e] = ch[e] * MSTRIDE + SING + 2 * c + (drank[e] % 2)

    m2slot = np.full(NE, -1, np.int64)
    m2pass = np.full(NE, -1, np.int64)
    for k in range(NDUPE):
        ki, kcnt = _cumcount(cellM, srank == k + 1)
        assert kcnt.size == 0 or kcnt.max() < M2CAP, f"M2 overflow {kcnt.max()+1}"
        m2slot[ki] = ch[ki] * M2CAP + kcnt
        m2pass[ki] = k

    c2 = np.full(NE, -1, np.int64)
    for k in range(NDUPE):
        ki, cko = _cumcount(p_d[sibling] * P + p_d, srank == k + 1)
        assert cko.size == 0 or cko.max() < C2, f"C2 overflow {cko.max()+1}"
        c2[ki] = cko

    # spill layers
    sym_of = t_d * R + r
    spill_ctr = {}
    pair_spill = {}
    for e in pi:
        kk = (p_d[e], sym_of[e])
        L = spill_ctr.get(kk, 0)
        assert L < NSPILL, "spill overflow"
        spill_ctr[kk] = L + 1
        pair_spill[(key_d[e], drank[e] // 2)] = L
    m2_spill = np.full(NE, -1, np.int64)
    for e in np.nonzero(srank >= 1)[0]:
        kk = (p_d[e], sym_of[e])
        L = spill_ctr.get(kk, 0)
        assert L < NSPILL, "spill overflow"
        spill_ctr[kk] = L + 1
        m2_spill[e] = L

    # ---- device arrays
    arr = {}
    idx1 = np.full((NG, P, GR * T), -1, np.int16)
    ma = np.nonzero(main)[0]
    idx1[g[ma], p_s[ma], (r[ma] % GR) * T + t_s[ma]] = \
        (c1 * P + p_d[mi]).astype(np.int16)
    arr["idx1"] = idx1

    idx2 = np.full((P, BLEN), -1, np.int16)
    idx2[p_d[mi], (g[mi] * C1 + c1) * P + p_s[mi]] = Mslot[mi].astype(np.int16)
    arr["idx2"] = idx2

    valM = np.zeros((P, NCH * MSTRIDE), np.float32)
    valM[p_d[mi], Mslot[mi]] = v[mi]
    arr["valM"] = valM.astype(bf16)

    idxd = np.full((NDUPE, P, NCH * MSTRIDE), -1, np.int16)
    idx2d = np.full((NDUPE, P, C2 * P), -1, np.int16)
    valM2 = np.zeros((NDUPE, P, NCH * M2CAP), np.float32)
    for k in range(NDUPE):
        ki = np.nonzero(srank == k + 1)[0]
        sib = sibling[ki]
        idxd[k, p_d[sib], Mslot[sib]] = (c2[ki] * P + p_d[ki]).astype(np.int16)
        idx2d[k, p_d[ki], c2[ki] * P + p_d[sib]] = m2slot[ki].astype(np.int16)
        valM2[k, p_d[ki], m2slot[ki]] = v[ki]
    arr["idxd"] = idxd
    arr["idx2d"] = idx2d
    arr["valM2"] = valM2.astype(bf16)

    idx3 = np.full((P, NCH * SING), -1, np.int16)
    idx3[p_d[si], ch[si] * SING + scnt] = \
        ((t_d[si] - ch[si] * TCH) * R + r[si]).astype(np.int16)
    arr["idx3"] = idx3

    idxsp = np.full((NSPILL, P, NCH * SPSTRIDE), -1, np.int16)
    plan = set()
    for e in pi:
        c = pairslot[(key_d[e], drank[e] // 2)]
        L = pair_spill[(key_d[e], drank[e] // 2)]
        idxsp[L, p_d[e], ch[e] * SPSTRIDE + c] = \
            (t_d[e] - ch[e] * TCH) * R + r[e]
        plan.add((L, int(ch[e])))
    for e in np.nonzero(srank >= 1)[0]:
        k = m2pass[e]
        L = m2_spill[e]
        rel = m2slot[e] - ch[e] * M2CAP
        idxsp[L, p_d[e], ch[e] * SPSTRIDE + PAIR + k * M2CAP + rel] = \
            (t_d[e] - ch[e] * TCH) * R + r[e]
        plan.add((int(L), int(ch[e])))
    arr["idxsp"] = idxsp

    arr["hv_sb"] = np.ascontiguousarray(
        hv_core.reshape(R, T, P).transpose(2, 0, 1).reshape(P, R * T)
    ).astype(bf16)
    return arr, plan


def _prep_core(core, head_vector, head_emb, pred_emb, edge_val, edge_src,
               edge_dst):
    b0 = core * BLOC
    hv = np.zeros((R, NPAD), np.float32)
    ev = np.empty((R, E), np.float32)
    es = np.empty((R, E), np.int64)
    ed = np.empty((R, E), np.int64)
    hemb = np.empty((R, D), np.float32)
    pemb = np.empty((R, D), np.float32)
    for a in range(A):
        sl = slice(a * BLOC, (a + 1) * BLOC)
        hv[sl, :N] = head_vector[a, b0:b0 + BLOC]
        ev[sl] = edge_val[a, b0:b0 + BLOC]
        es[sl] = edge_src[a, b0:b0 + BLOC]
        ed[sl] = edge_dst[a, b0:b0 + BLOC]
        hemb[sl] = head_emb[a, b0:b0 + BLOC]
        pemb[sl] = pred_emb[a, b0:b0 + BLOC]

    arr, plan = _prep_symbolic(hv, es, ed, ev)
    arr["hemb"] = hemb
    arr["pemb"] = pemb
    return arr, plan


def _build_host_inputs(entity_embedding, head_vector, head_emb, pred_emb,
                       edge_val, edge_src, edge_dst):
    global _SPILL_PLAN
    entity_pad = np.zeros((NPAD, D), np.float32)
    entity_pad[:N] = entity_embedding
    # proj moving operand entN: [g, p, k*512+dd] = entity[(8g+k)*128+p, dd]
    npadg = ((T + 7) // 8) * 8 * P
    en_t = np.zeros((npadg // P, P, D), np.float32)
    en_t[:T] = entity_pad.reshape(T, P, D)
    entN = np.ascontiguousarray(
        en_t.reshape(-1, 8, P, D).transpose(0, 2, 1, 3).reshape(-1, P, 8 * D)
    ).astype(bf16)
    # score stationary entT: [g, p, k*512+dk*128+j] = entity[(13g+k)*128+j, dk*128+p]
    ngrp = (T + NWIN - 1) // NWIN
    et_t = np.zeros((ngrp * NWIN, P, D), np.float32)
    et_t[:T] = entity_pad.reshape(T, P, D)
    et5 = et_t.reshape(ngrp, NWIN, P, 4, P).transpose(0, 4, 1, 3, 2)
    entT = np.ascontiguousarray(et5.reshape(ngrp, P, NWIN * D)).astype(bf16)
    ident = np.eye(P, dtype=np.float32)
    identb = np.eye(P, dtype=bf16)
    ones128 = np.ones((P, 1), np.float32)
    padm = np.zeros((P, 1), np.float32)
    padm[N - (T - 1) * P:] = -1e30

    in_maps = []
    plan_all = set()
    for core in range(NCORES):
        m, plan = _prep_core(core, head_vector, head_emb, pred_emb,
                             edge_val, edge_src, edge_dst)
        plan_all |= plan
        m["entT"] = entT
        m["entN"] = entN
        m["ident"] = ident
        m["identb"] = identb
        m["ones128"] = ones128
        m["padm"] = padm
        in_maps.append(m)
    _SPILL_PLAN = sorted(plan_all)
    return in_maps


# ---------------------------------------------------------------------------
# Bass program
# ---------------------------------------------------------------------------

def build_program():
    from contextlib import ExitStack
    import concourse.bass as bass
    import concourse.tile as tile
    from concourse import bacc, mybir
    from concourse.alu_op_type import AluOpType as op
    import bass_rust

    dt = mybir.dt
    f32, b16, i16 = dt.float32, dt.bfloat16, dt.int16
    Exp = bass_rust.ActivationFunctionType.Exp

    nc = bacc.Bacc("TRN2", target_bir_lowering=False, debug=False,
                   num_devices=NCORES)

    def din(name, shape, dtype):
        return nc.dram_tensor(name, list(shape), dtype, kind="ExternalInput").ap()

    NGRP = (T + NWIN - 1) // NWIN
    NPGRP = (T + 7) // 8
    entT = din("entT", (NGRP, P, NWIN * D), b16)
    entN = din("entN", (NPGRP, P, 8 * D), b16)
    hemb = din("hemb", (R, D), f32)
    pemb = din("pemb", (R, D), f32)
    ident = din("ident", (P, P), f32)
    identb = din("identb", (P, P), b16)
    ones128 = din("ones128", (P, 1), f32)
    padm = din("padm", (P, 1), f32)
    hv_sb_d = din("hv_sb", (P, R * T), b16)
    idx1_d = din("idx1", (NG, P, GR * T), i16)
    idx2_d = din("idx2", (P, BLEN), i16)
    valM_d = din("valM", (P, NCH * MSTRIDE), b16)
    idxd_d = din("idxd", (NDUPE, P, NCH * MSTRIDE), i16)
    idx2d_d = din("idx2d", (NDUPE, P, C2 * P), i16)
    valM2_d = din("valM2", (NDUPE, P, NCH * M2CAP), b16)
    idx3_d = din("idx3", (P, NCH * SING), i16)
    idxsp_d = din("idxsp", (NSPILL, P, NCH * SPSTRIDE), i16)

    out = nc.dram_tensor("out", [BLOC, D], f32, kind="ExternalOutput").ap()

    with tile.TileContext(nc) as tc, ExitStack() as ctx:
        ctx.enter_context(nc.allow_low_precision(
            reason="bf16 storage is deliberate; reductions accumulate f32"))
        const = ctx.enter_context(tc.tile_pool(name="const", bufs=1))
        big = ctx.enter_context(tc.tile_pool(name="big", bufs=1))
        small = ctx.enter_context(tc.tile_pool(name="small", bufs=1))
        sym = ctx.enter_context(tc.tile_pool(name="sym", bufs=1))
        symi = ctx.enter_context(tc.tile_pool(name="symi", bufs=2))
        entp = ctx.enter_context(tc.tile_pool(name="entp", bufs=2))
        ps_score = ctx.enter_context(tc.tile_pool(name="ps_score", bufs=2, space="PSUM"))
        ps_tr = ctx.enter_context(tc.tile_pool(name="ps_tr", bufs=1, space="PSUM"))
        ps_trb = ctx.enter_context(tc.tile_pool(name="ps_trb", bufs=2, space="PSUM"))
        ps_proj = ctx.enter_context(tc.tile_pool(name="ps_proj", bufs=1, space="PSUM"))
        early = ExitStack()
        early1 = early.enter_context(tc.tile_pool(name="early1", bufs=2))
        earlys = early.enter_context(tc.tile_pool(name="earlys", bufs=4))
        earlyb = early.enter_context(tc.tile_pool(name="earlyb", bufs=1))
        enpool = []   # created after early pools close

        # ---- constants
        ident_sb = const.tile([P, P], f32)
        nc.sync.dma_start(ident_sb[:], ident[:])
        identb_sb = const.tile([P, P], b16)
        nc.sync.dma_start(identb_sb[:], identb[:])
        ones128_sb = const.tile([P, 1], f32)
        nc.sync.dma_start(ones128_sb[:], ones128[:])
        padm_sb = const.tile([P, 1], f32)
        nc.sync.dma_start(padm_sb[:], padm[:])

        # ---- symbolic small input DMAs (scalar-engine DMA queue)
        idx2_t = earlyb.tile([P, BLEN], i16, tag="idx2")
        nc.scalar.dma_start(idx2_t[:], idx2_d[:])
        valM_t = sym.tile([P, NCH * MSTRIDE], b16, tag="valM")
        nc.scalar.dma_start(valM_t[:], valM_d[:])
        idx2d_t = sym.tile([P, NDUPE, C2 * P], i16, tag="idx2d")
        nc.scalar.dma_start(idx2d_t[:], idx2d_d[:].rearrange("k p n -> p k n"))
        valM2_t = sym.tile([P, NDUPE, NCH * M2CAP], b16, tag="valM2")
        nc.scalar.dma_start(valM2_t[:], valM2_d[:].rearrange("k p n -> p k n"))
        idx3_t = sym.tile([P, NCH * SING], i16, tag="idx3")
        nc.scalar.dma_start(idx3_t[:], idx3_d[:])
        idxsp_t = sym.tile([P, NSPILL, NCH * SPSTRIDE], i16, tag="idxsp")
        nc.scalar.dma_start(idxsp_t[:], idxsp_d[:].rearrange("k p n -> p k n"))

        # ---- tail = head + pred -> tailT [128d, dk*32+r] bf16
        tail_f = small.tile([R, D], f32, tag="tailf")
        h_t = small.tile([R, D], f32, tag="hp")
        p_t = small.tile([R, D], f32, tag="hp2")
        nc.sync.dma_start(h_t[:], hemb[:])
        nc.sync.dma_start(p_t[:], pemb[:])
        nc.vector.tensor_tensor(out=tail_f[:], in0=h_t[:], in1=p_t[:], op=op.add)
        tailT = const.tile([P, 4 * R], b16)
        for dk in range(4):
            pt = ps_tr.tile([P, 512], f32, tag="tr")
            nc.tensor.transpose(out=pt[:, :R], in_=tail_f[:, dk * P:(dk + 1) * P],
                                identity=ident_sb[:R, :R])
            nc.vector.tensor_copy(out=tailT[:, dk * R:(dk + 1) * R], in_=pt[:, :R])

        # ---- symbolic stage 1: 4 x (dma hv slice + local_scatter)
        stg = []
        for g in range(NG):
            hvg = early1.tile([P, GR * T], b16, tag="hvg")
            nc.scalar.dma_start(hvg[:], hv_sb_d[:, g * GR * T:(g + 1) * GR * T])
            i1 = early1.tile([P, GR * T], i16, tag="idx1")
            nc.scalar.dma_start(i1[:], idx1_d[g])
            st = earlys.tile([P, C1 * P], b16, tag="stg")
            nc.gpsimd.local_scatter(st[:], hvg[:], i1[:], channels=P,
                                    num_elems=C1 * P, num_idxs=GR * T)
            stg.append(st)

        Bt = earlyb.tile([P, BLEN], b16, tag="B")
        M_t = sym.tile([P, NCH * MSTRIDE], b16, tag="M")
        M2 = []
        spill = sym.tile([P, NCH, SPSTRIDE], b16, tag="spill")
        sym3 = big.tile([P, T, R], b16, tag="bigB")
        sym3f = sym3[:].rearrange("p t r -> p (t r)")

        def do_b_transposes():
            for g in range(NG):
                for c in range(C1):
                    pt = ps_trb.tile([P, 512], b16, tag="trb")
                    nc.tensor.transpose(out=pt[:, :P],
                                        in_=stg[g][:, c * P:(c + 1) * P],
                                        identity=identb_sb[:])
                    nc.vector.tensor_copy(
                        out=Bt[:, (g * C1 + c) * P:(g * C1 + c + 1) * P],
                        in_=pt[:, :P])
            nc.gpsimd.local_scatter(M_t[:], Bt[:], idx2_t[:], channels=P,
                                    num_elems=NCH * MSTRIDE, num_idxs=BLEN)

        def do_dupe_pass(k):
            idxd_t = symi.tile([P, NCH * MSTRIDE], i16, tag="idxd")
            nc.scalar.dma_start(idxd_t[:], idxd_d[k])
            st2 = symi.tile([P, C2 * P], b16, tag="stg2")
            nc.gpsimd.local_scatter(st2[:], M_t[:], idxd_t[:], channels=P,
                                    num_elems=C2 * P, num_idxs=NCH * MSTRIDE)
            Bd = symi.tile([P, C2 * P], b16, tag="Bd")
            for c in range(C2):
                pt = ps_trb.tile([P, 512], b16, tag="trb")
                nc.tensor.transpose(out=pt[:, :P], in_=st2[:, c * P:(c + 1) * P],
                                    identity=identb_sb[:])
                nc.vector.tensor_copy(out=Bd[:, c * P:(c + 1) * P], in_=pt[:, :P])
            m2t = sym.tile([P, NCH * M2CAP], b16, tag=f"M2_{k}")
            nc.gpsimd.local_scatter(m2t[:], Bd[:], idx2d_t[:, k, :], channels=P,
                                    num_elems=NCH * M2CAP, num_idxs=C2 * P)
            M2.append(m2t)

        def close_early():
            early.close()
            enp = ctx.enter_context(tc.tile_pool(name="enp", bufs=3))
            enpool.append(enp)

        def do_symbolic_finish():
            nc.vector.tensor_tensor(out=M_t[:], in0=M_t[:], in1=valM_t[:],
                                    op=op.mult)
            a = M_t[:]
            in0 = bass.AP(a.tensor, a.offset + SING,
                          [list(a.ap[0]), [MSTRIDE, NCH], [2, PAIR]])
            in1 = bass.AP(a.tensor, a.offset + SING + 1,
                          [list(a.ap[0]), [MSTRIDE, NCH], [2, PAIR]])
            nc.vector.tensor_tensor(out=spill[:, :, :PAIR], in0=in0, in1=in1,
                                    op=op.add)
            for k in range(NDUPE):
                m2v = M2[k][:].rearrange("p (ch s) -> p ch s", s=M2CAP)
                nc.vector.tensor_tensor(
                    out=m2v, in0=m2v,
                    in1=valM2_t[:, k, :].rearrange("p (ch s) -> p ch s", s=M2CAP),
                    op=op.mult)
                nc.vector.tensor_copy(
                    out=spill[:, :, PAIR + k * M2CAP:PAIR + (k + 1) * M2CAP],
                    in_=m2v)
            for c in range(NCH):
                ne = _chunk_elems(c)
                nc.gpsimd.local_scatter(
                    sym3f[:, c * TCH * R: c * TCH * R + ne],
                    M_t[:, c * MSTRIDE:c * MSTRIDE + SING],
                    idx3_t[:, c * SING:(c + 1) * SING],
                    channels=P, num_elems=ne, num_idxs=SING)
            for (L, c) in _SPILL_PLAN:
                ne = _chunk_elems(c)
                ssp = sym.tile([P, TCH * R], b16, tag="ssp")
                nc.gpsimd.local_scatter(
                    ssp[:, :ne], spill[:, c, :],
                    idxsp_t[:, L, c * SPSTRIDE:(c + 1) * SPSTRIDE],
                    channels=P, num_elems=ne, num_idxs=SPSTRIDE)
                nc.vector.tensor_tensor(
                    out=sym3f[:, c * TCH * R: c * TCH * R + ne],
                    in0=sym3f[:, c * TCH * R: c * TCH * R + ne],
                    in1=ssp[:, :ne], op=op.add)

        # ---- helpers
        def col_reduce_bcast(x_rc, nr, op_red, post, tagp):
            pt = ps_tr.tile([P, 512], f32, tag="tr")
            nc.tensor.transpose(out=pt[:nr, :P], in_=x_rc, identity=ident_sb[:])
            v = small.tile([nr, 1], f32, tag="v" + tagp)
            nc.vector.tensor_reduce(out=v[:], in_=pt[:nr, :P],
                                    axis=mybir.AxisListType.X, op=op_red)
            v2 = small.tile([nr, 1], b16, tag="w" + tagp)
            post(v, v2)
            pb = ps_trb.tile([P, 512], b16, tag="trb")
            vb = v2[:].to_broadcast([nr, P])
            nc.tensor.transpose(out=pb[:, :nr], in_=vb, identity=identb_sb[:nr, :nr])
            return pb

        def bcast8(ps_tile, r0, cnt8, nr):
            a = ps_tile[:, r0:r0 + nr]
            return bass.AP(a.tensor, a.offset,
                           [list(a.ap[0]), [0, cnt8], list(a.ap[1])])

        sy_flat = sym3[:].rearrange("p t r -> p (t r)")
        sy_rt = sym3[:].rearrange("p t r -> p r t")

        y1 = small.tile([P, R], f32, tag="y1")

        def do_y1():
            nc.vector.tensor_reduce(out=y1[:], in_=sy_rt,
                                    axis=mybir.AxisListType.X, op=op.add)

        hooks = {10: do_b_transposes,
                 12: close_early,
                 16: lambda: do_dupe_pass(0),
                 17: lambda: do_dupe_pass(1),
                 18: lambda: do_dupe_pass(2),
                 20: do_symbolic_finish,
                 27: do_y1}

        # ---- score: per group: matmuls -> copy -> (mask) -> exp -> s1 partial
        score3 = big.tile([P, T, R], b16, tag="bigA")
        s1p = small.tile([P, NGRP, R], f32, tag="s1p")
        for grp in range(NGRP):
            if grp in hooks:
                hooks[grp]()
            t0 = grp * NWIN
            cnt = min(NWIN, T - t0)
            et = entp.tile([P, NWIN, 4, P], b16, tag="et")
            nc.sync.dma_start(et[:], entT[grp])
            pss = ps_score.tile([P, TPAD], f32, tag="ps_s")
            for k in range(cnt):
                for dk in range(4):
                    nc.tensor.matmul(out=pss[:, 32 * k:32 * k + 32],
                                     lhsT=et[:, k, dk, :],
                                     rhs=tailT[:, dk * R:(dk + 1) * R],
                                     start=(dk == 0), stop=(dk == 3))
            nc.vector.tensor_copy(out=score3[:, t0:t0 + cnt, :],
                                  in_=pss[:, :32 * cnt])
            if grp == NGRP - 1:
                nc.vector.tensor_scalar(out=score3[:, T - 1:T, :],
                                        in0=score3[:, T - 1:T, :],
                                        scalar1=padm_sb[:], scalar2=None,
                                        op0=op.add)
            sl_flat = score3[:, t0:t0 + cnt, :].rearrange("p t r -> p (t r)")
            nc.scalar.activation(out=sl_flat, in_=sl_flat, func=Exp)
            nc.vector.tensor_reduce(
                out=s1p[:, grp, :],
                in_=score3[:, t0:t0 + cnt, :].rearrange("p t r -> p r t"),
                axis=mybir.AxisListType.X, op=op.add)

        # ---- normalizers: sB = 1/sum(exp), yB = 1/max(CLIP, sum(sym))
        s1 = small.tile([P, R], f32, tag="s1")
        nc.vector.tensor_reduce(out=s1[:], in_=s1p[:].rearrange("p g r -> p r g"),
                                axis=mybir.AxisListType.X, op=op.add)

        def recip_post(v, v2):
            nc.vector.reciprocal(out=v2[:], in_=v[:])

        def clipmax_recip(v, v2):
            nc.vector.tensor_scalar(out=v[:], in0=v[:], scalar1=float(CLIP),
                                    scalar2=None, op0=op.max)
            nc.vector.reciprocal(out=v2[:], in_=v[:])

        sB = col_reduce_bcast(s1[:], R, op.add, recip_post, "s")
        yB = col_reduce_bcast(y1[:], R, op.add, clipmax_recip, "y")

        def bcast_full(ps_tile, nr):
            a = ps_tile[:, :nr]
            return bass.AP(a.tensor, a.offset,
                           [list(a.ap[0]), [0, T], list(a.ap[1])])

        sc_flat = score3[:].rearrange("p t r -> p (t r)")
        # prescale in place: score3 <- neural (normalized), sym3 <- sym_n
        nc.vector.tensor_tensor(out=sc_flat, in0=sc_flat, in1=bcast_full(sB, R),
                                op=op.mult)
        nc.vector.tensor_tensor(out=sy_flat, in0=sy_flat, in1=bcast_full(yB, R),
                                op=op.mult)

        # ---- agg_b = (sym0'+e0')*(sym1'+e1') in 6 big chunks
        agg_b = big.tile([P, T, BLOC], b16, tag="aggb")
        ACH = 66
        ft1 = small.tile([P, ACH, BLOC], b16, tag="ft1")
        ft2 = small.tile([P, ACH, BLOC], b16, tag="ft2")
        for c0 in range(0, T, ACH):
            cc = min(ACH, T - c0)
            sl = slice(c0, c0 + cc)
            nc.vector.tensor_tensor(out=ft1[:, :cc], in0=sym3[:, sl, :BLOC],
                                    in1=score3[:, sl, :BLOC], op=op.add)
            nc.vector.tensor_tensor(out=ft2[:, :cc], in0=sym3[:, sl, BLOC:],
                                    in1=score3[:, sl, BLOC:], op=op.add)
            nc.vector.tensor_tensor(out=agg_b[:, sl, :], in0=ft1[:, :cc],
                                    in1=ft2[:, :cc], op=op.mult)

        # ---- projection loop: pure DMA + matmul
        pso = ps_proj.tile([BLOC, D], f32)
        enp = enpool[0]
        for g in range(NPGRP):
            nt0 = 8 * g
            cnt8 = min(8, T - nt0)
            en = enp.tile([P, 8, D], b16, tag="en")
            nc.sync.dma_start(en[:], entN[g])
            for k in range(cnt8):
                nt = nt0 + k
                nc.tensor.matmul(out=pso[:], lhsT=agg_b[:, nt, :],
                                 rhs=en[:, k, :],
                                 start=(nt == 0), stop=(nt == T - 1))

        # ---- denominator: sum over n of agg_unnorm, then out = pso / denom
        g1 = small.tile([P, BLOC], f32, tag="g1")
        nc.vector.tensor_reduce(out=g1[:],
                                in_=agg_b[:].rearrange("p t r -> p r t"),
                                axis=mybir.AxisListType.X, op=op.add)
        pd = ps_tr.tile([P, 512], f32, tag="tr")
        nc.tensor.matmul(out=pd[:BLOC, :1], lhsT=g1[:], rhs=ones128_sb[:],
                         start=True, stop=True)
        dclip = small.tile([BLOC, 1], f32, tag="dclip")
        nc.vector.tensor_scalar(out=dclip[:], in0=pd[:BLOC, :1],
                                scalar1=float(CLIP), scalar2=None, op0=op.max)
        drcp = small.tile([BLOC, 1], f32, tag="drcp")
        nc.vector.reciprocal(out=drcp[:], in_=dclip[:])
        out_sb = small.tile([BLOC, D], f32, tag="outsb")
        nc.vector.tensor_scalar(out=out_sb[:], in0=pso[:], scalar1=drcp[:],
                                scalar2=None, op0=op.mult)
        nc.sync.dma_start(out[:], out_sb[:])

    nc.compile()
    return nc


_PROGRAM = None


def kernel(entity_embedding, head_vector, head_emb, pred_emb,
           edge_val, edge_src, edge_dst):
    global _PROGRAM
    from concourse.bass_utils import run_bass_kernel_spmd

    in_maps = _build_host_inputs(entity_embedding, head_vector,
                                 head_emb, pred_emb,
                                 edge_val, edge_src, edge_dst)
    if _PROGRAM is None:
        _PROGRAM = build_program()
    res = run_bass_kernel_spmd(_PROGRAM, in_maps, list(range(NCORES)))
    out = np.empty((B, D), np.float32)
    for c in range(NCORES):
        out[c * BLOC:(c + 1) * BLOC] = res.results[c]["out"]
    return out


if __name__ == "__main__":
    import reference
    inputs = {k: np.asarray(v) for k, v in reference.setup_inputs().items()}
    got = kernel(**inputs)
    want = np.asarray(reference.reference(**inputs))
    err = np.abs(got - want).max() / np.abs(want).max()
    print("Relative error:", err)


# revision 9
# speedup vs baseline: 3.8131x; 1.0091x over previous
"""Trainium2 Bass kernel for NeuralSymbolicMP layer (gnn_message_passing).

Batch-sharded over B across 8 NeuronCores; each core handles 32 (atomic,
batch) rows x all N entities.

Symbolic message passing runs as a GpSimd local_scatter permutation network:
  hv (src-partition layout) --local_scatter--> staging --PE transpose-->
  B (dst-partition layout) --local_scatter--> M (per-edge, dst-ordered)
  --x val--> --local_scatter--> dense sym3 [128, T, R]
with duplicate-src edges resolved by sibling-read passes over M and
duplicate-dst edges by pair slots + spill layers.

Neural side (score matmul, softmax, clip-norm chain, projection) is a dense
PE/DVE pipeline over entity tiles streamed from HBM in bf16.
"""

import numpy as np
import ml_dtypes

A, B, N, D, E = 2, 128, 50000, 512, 4096
CLIP = 1e-14

NCORES = 8
BLOC = B // NCORES          # 16 batch per core
R = A * BLOC                # 32 rows per core
P = 128
T = 391                     # n tiles: 391*128 = 50048
NPAD = T * P                # 50048
NWIN = 13                   # score psum windows of 32 tiles
TPAD = NWIN * 32            # 416

# symbolic permutation network constants
NG = 4                      # row groups for staging
GR = R // NG                # 8 rows per group
C1 = 13                     # staging chunks per group
TCH = 56                    # t-tiles per M chunk
NCH = 7                     # M chunks
SING = 224                  # singles per (p, chunk)
PAIR = 16                   # pair slots per (p, chunk)
MSTRIDE = SING + 2 * PAIR   # 256
C2 = 6                      # dupe staging chunks
M2CAP = 24                  # M2 capacity per (p, chunk)
NDUPE = 3                   # dupe passes
SPSTRIDE = PAIR + NDUPE * M2CAP  # 88
NSPILL = 3                  # spill layers
SYM_LEN = T * R             # 12512
BLEN = NG * C1 * P          # 6656

bf16 = ml_dtypes.bfloat16

# spill build plan: set of (layer, chunk) with any entry on any core.
# populated by _build_host_inputs before build_program is called.
_SPILL_PLAN = [(L, c) for L in range(NSPILL) for c in range(NCH)]


def _chunk_elems(ch):
    t0, t1 = ch * TCH, min((ch + 1) * TCH, T)
    return (t1 - t0) * R


def _cumcount(keys, sel):
    """For each selected element, its 0-based rank among same-key elements."""
    idxs = np.nonzero(sel)[0]
    if idxs.size == 0:
        return idxs, np.zeros(0, np.int64)
    kk = keys[idxs]
    o = np.argsort(kk, kind="stable")
    k = kk[o]
    n = np.ones(idxs.size, bool)
    n[1:] = k[1:] != k[:-1]
    g = np.cumsum(n) - 1
    f = np.zeros(g.max() + 1, np.int64)
    f[g[n]] = np.nonzero(n)[0]
    cnt_sorted = np.arange(idxs.size) - f[g]
    out = np.empty(idxs.size, np.int64)
    out[o] = cnt_sorted
    return idxs, out


def _prep_symbolic(hv_core, src, dst, val):
    """hv_core [R, NPAD] f32, src/dst/val [R, E]. Returns device arrays +
    spill plan entries."""
    r = np.repeat(np.arange(R), E)
    s = src.ravel().astype(np.int64)
    d = dst.ravel().astype(np.int64)
    v = val.ravel().astype(np.float32)
    NE = r.size

    p_s, t_s = s % P, s // P
    p_d, t_d = d % P, d // P
    g = r // GR
    ch = t_d // TCH

    # src-group ranks
    key_s = r * NPAD + s
    o = np.argsort(key_s, kind="stable")
    ks = key_s[o]
    newg = np.ones(NE, bool)
    newg[1:] = ks[1:] != ks[:-1]
    gid = np.cumsum(newg) - 1
    first = np.zeros(gid.max() + 1, np.int64)
    first[gid[newg]] = np.nonzero(newg)[0]
    srank = np.empty(NE, np.int64)
    srank[o] = np.arange(NE) - first[gid]
    assert srank.max() <= NDUPE, f"src dupe depth {srank.max()}"
    main = srank == 0
    sibling = np.empty(NE, np.int64)
    sibling[o] = o[first[gid]]

    # staging chunks for main edges
    mi, c1 = _cumcount((g * P + p_s) * P + p_d, main)
    assert c1.max() < C1, f"C1 overflow {c1.max() + 1}"

    # dst groups: mains first
    key_d = r * NPAD + d
    od = np.argsort(key_d * 8 + np.minimum(srank, 7), kind="stable")
    kd = key_d[od]
    ngd = np.ones(NE, bool)
    ngd[1:] = kd[1:] != kd[:-1]
    gidd = np.cumsum(ngd) - 1
    fd = np.zeros(gidd.max() + 1, np.int64)
    fd[gidd[ngd]] = np.nonzero(ngd)[0]
    drank = np.empty(NE, np.int64)
    drank[od] = np.arange(NE) - fd[gidd]
    gmain = np.zeros(gidd.max() + 1, np.int64)
    np.add.at(gmain, gidd, main[od].astype(np.int64))
    qmain = np.empty(NE, np.int64)
    qmain[od] = gmain[gidd]

    is_single = main & (qmain == 1)
    is_pairm = main & (qmain >= 2)
    cellM = p_d * NCH + ch

    si, scnt = _cumcount(cellM, is_single)
    assert scnt.size == 0 or scnt.max() < SING, f"SING overflow {scnt.max()+1}"
    pair_lead = is_pairm & (drank % 2 == 0)
    pi, pcnt = _cumcount(cellM, pair_lead)
    assert pcnt.size == 0 or pcnt.max() < PAIR, f"PAIR overflow {pcnt.max()+1}"
    pairslot = {(key_d[e], drank[e] // 2): c for e, c in zip(pi, pcnt)}

    Mslot = np.full(NE, -1, np.int64)
    Mslot[si] = ch[si] * MSTRIDE + scnt
    pm = np.nonzero(is_pairm)[0]
    for e in pm:
        c = pairslot[(key_d[e], drank[e] // 2)]
        Mslot[e] = ch[e] * MSTRIDE + SING + 2 * c + (drank[e] % 2)

    m2slot = np.full(NE, -1, np.int64)
    m2pass = np.full(NE, -1, np.int64)
    for k in range(NDUPE):
        ki, kcnt = _cumcount(cellM, srank == k + 1)
        assert kcnt.size == 0 or kcnt.max() < M2CAP, f"M2 overflow {kcnt.max()+1}"
        m2slot[ki] = ch[ki] * M2CAP + kcnt
        m2pass[ki] = k

    c2 = np.full(NE, -1, np.int64)
    for k in range(NDUPE):
        ki, cko = _cumcount(p_d[sibling] * P + p_d, srank == k + 1)
        assert cko.size == 0 or cko.max() < C2, f"C2 overflow {cko.max()+1}"
        c2[ki] = cko

    # spill layers
    sym_of = t_d * R + r
    spill_ctr = {}
    pair_spill = {}
    for e in pi:
        kk = (p_d[e], sym_of[e])
        L = spill_ctr.get(kk, 0)
        assert L < NSPILL, "spill overflow"
        spill_ctr[kk] = L + 1
        pair_spill[(key_d[e], drank[e] // 2)] = L
    m2_spill = np.full(NE, -1, np.int64)
    for e in np.nonzero(srank >= 1)[0]:
        kk = (p_d[e], sym_of[e])
        L = spill_ctr.get(kk, 0)
        assert L < NSPILL, "spill overflow"
        spill_ctr[kk] = L + 1
        m2_spill[e] = L

    # ---- device arrays
    arr = {}
    idx1 = np.full((NG, P, GR * T), -1, np.int16)
    ma = np.nonzero(main)[0]
    idx1[g[ma], p_s[ma], (r[ma] % GR) * T + t_s[ma]] = \
        (c1 * P + p_d[mi]).astype(np.int16)
    arr["idx1"] = idx1

    idx2 = np.full((P, BLEN), -1, np.int16)
    idx2[p_d[mi], (g[mi] * C1 + c1) * P + p_s[mi]] = Mslot[mi].astype(np.int16)
    arr["idx2"] = idx2

    valM = np.zeros((P, NCH * MSTRIDE), np.float32)
    valM[p_d[mi], Mslot[mi]] = v[mi]
    arr["valM"] = valM.astype(bf16)

    idxd = np.full((NDUPE, P, NCH * MSTRIDE), -1, np.int16)
    idx2d = np.full((NDUPE, P, C2 * P), -1, np.int16)
    valM2 = np.zeros((NDUPE, P, NCH * M2CAP), np.float32)
    for k in range(NDUPE):
        ki = np.nonzero(srank == k + 1)[0]
        sib = sibling[ki]
        idxd[k, p_d[sib], Mslot[sib]] = (c2[ki] * P + p_d[ki]).astype(np.int16)
        idx2d[k, p_d[ki], c2[ki] * P + p_d[sib]] = m2slot[ki].astype(np.int16)
        valM2[k, p_d[ki], m2slot[ki]] = v[ki]
    arr["idxd"] = idxd
    arr["idx2d"] = idx2d
    arr["valM2"] = valM2.astype(bf16)

    idx3 = np.full((P, NCH * SING), -1, np.int16)
    idx3[p_d[si], ch[si] * SING + scnt] = \
        ((t_d[si] - ch[si] * TCH) * R + r[si]).astype(np.int16)
    arr["idx3"] = idx3

    idxsp = np.full((NSPILL, P, NCH * SPSTRIDE), -1, np.int16)
    plan = set()
    for e in pi:
        c = pairslot[(key_d[e], drank[e] // 2)]
        L = pair_spill[(key_d[e], drank[e] // 2)]
        idxsp[L, p_d[e], ch[e] * SPSTRIDE + c] = \
            (t_d[e] - ch[e] * TCH) * R + r[e]
        plan.add((L, int(ch[e])))
    for e in np.nonzero(srank >= 1)[0]:
        k = m2pass[e]
        L = m2_spill[e]
        rel = m2slot[e] - ch[e] * M2CAP
        idxsp[L, p_d[e], ch[e] * SPSTRIDE + PAIR + k * M2CAP + rel] = \
            (t_d[e] - ch[e] * TCH) * R + r[e]
        plan.add((int(L), int(ch[e])))
    arr["idxsp"] = idxsp

    arr["hv_sb"] = np.ascontiguousarray(
        hv_core.reshape(R, T, P).transpose(2, 0, 1).reshape(P, R * T)
    ).astype(bf16)
    return arr, plan


def _prep_core(core, head_vector, head_emb, pred_emb, edge_val, edge_src,
               edge_dst):
    b0 = core * BLOC
    hv = np.zeros((R, NPAD), np.float32)
    ev = np.empty((R, E), np.float32)
    es = np.empty((R, E), np.int64)
    ed = np.empty((R, E), np.int64)
    hemb = np.empty((R, D), np.float32)
    pemb = np.empty((R, D), np.float32)
    for a in range(A):
        sl = slice(a * BLOC, (a + 1) * BLOC)
        hv[sl, :N] = head_vector[a, b0:b0 + BLOC]
        ev[sl] = edge_val[a, b0:b0 + BLOC]
        es[sl] = edge_src[a, b0:b0 + BLOC]
        ed[sl] = edge_dst[a, b0:b0 + BLOC]
        hemb[sl] = head_emb[a, b0:b0 + BLOC]
        pemb[sl] = pred_emb[a, b0:b0 + BLOC]

    arr, plan = _prep_symbolic(hv, es, ed, ev)
    arr["hemb"] = hemb
    arr["pemb"] = pemb
    return arr, plan


def _build_host_inputs(entity_embedding, head_vector, head_emb, pred_emb,
                       edge_val, edge_src, edge_dst):
    global _SPILL_PLAN
    entity_pad = np.zeros((NPAD, D), np.float32)
    entity_pad[:N] = entity_embedding
    # proj moving operand entN: [g, p, k*512+dd] = entity[(8g+k)*128+p, dd]
    npadg = ((T + 15) // 16) * 16 * P
    en_t = np.zeros((npadg // P, P, D), np.float32)
    en_t[:T] = entity_pad.reshape(T, P, D)
    entN = np.ascontiguousarray(
        en_t.reshape(-1, 16, P, D).transpose(0, 2, 1, 3).reshape(-1, P, 16 * D)
    ).astype(bf16)
    # score stationary entT: [g, p, k*512+dk*128+j] = entity[(13g+k)*128+j, dk*128+p]
    ngrp = (T + NWIN - 1) // NWIN
    et_t = np.zeros((ngrp * NWIN, P, D), np.float32)
    et_t[:T] = entity_pad.reshape(T, P, D)
    et5 = et_t.reshape(ngrp, NWIN, P, 4, P).transpose(0, 4, 1, 3, 2)
    entT = np.ascontiguousarray(et5.reshape(ngrp, P, NWIN * D)).astype(bf16)
    ident = np.eye(P, dtype=np.float32)
    identb = np.eye(P, dtype=bf16)
    ones128 = np.ones((P, 1), np.float32)
    padm = np.zeros((P, 1), np.float32)
    padm[N - (T - 1) * P:] = -1e30

    in_maps = []
    plan_all = set()
    for core in range(NCORES):
        m, plan = _prep_core(core, head_vector, head_emb, pred_emb,
                             edge_val, edge_src, edge_dst)
        plan_all |= plan
        m["entT"] = entT
        m["entN"] = entN
        m["ident"] = ident
        m["identb"] = identb
        m["ones128"] = ones128
        m["padm"] = padm
        in_maps.append(m)
    _SPILL_PLAN = sorted(plan_all)
    return in_maps


# ---------------------------------------------------------------------------
# Bass program
# ---------------------------------------------------------------------------

def build_program():
    from contextlib import ExitStack
    import concourse.bass as bass
    import concourse.tile as tile
    from concourse import bacc, mybir
    from concourse.alu_op_type import AluOpType as op
    import bass_rust

    dt = mybir.dt
    f32, b16, i16 = dt.float32, dt.bfloat16, dt.int16
    Exp = bass_rust.ActivationFunctionType.Exp

    nc = bacc.Bacc("TRN2", target_bir_lowering=False, debug=False,
                   num_devices=NCORES)

    def din(name, shape, dtype):
        return nc.dram_tensor(name, list(shape), dtype, kind="ExternalInput").ap()

    NGRP = (T + NWIN - 1) // NWIN
    NPGRP = (T + 15) // 16
    entT = din("entT", (NGRP, P, NWIN * D), b16)
    entN = din("entN", (NPGRP, P, 16 * D), b16)
    hemb = din("hemb", (R, D), f32)
    pemb = din("pemb", (R, D), f32)
    ident = din("ident", (P, P), f32)
    identb = din("identb", (P, P), b16)
    ones128 = din("ones128", (P, 1), f32)
    padm = din("padm", (P, 1), f32)
    hv_sb_d = din("hv_sb", (P, R * T), b16)
    idx1_d = din("idx1", (NG, P, GR * T), i16)
    idx2_d = din("idx2", (P, BLEN), i16)
    valM_d = din("valM", (P, NCH * MSTRIDE), b16)
    idxd_d = din("idxd", (NDUPE, P, NCH * MSTRIDE), i16)
    idx2d_d = din("idx2d", (NDUPE, P, C2 * P), i16)
    valM2_d = din("valM2", (NDUPE, P, NCH * M2CAP), b16)
    idx3_d = din("idx3", (P, NCH * SING), i16)
    idxsp_d = din("idxsp", (NSPILL, P, NCH * SPSTRIDE), i16)

    out = nc.dram_tensor("out", [BLOC, D], f32, kind="ExternalOutput").ap()

    with tile.TileContext(nc) as tc, ExitStack() as ctx:
        ctx.enter_context(nc.allow_low_precision(
            reason="bf16 storage is deliberate; reductions accumulate f32"))
        const = ctx.enter_context(tc.tile_pool(name="const", bufs=1))
        big = ctx.enter_context(tc.tile_pool(name="big", bufs=1))
        small = ctx.enter_context(tc.tile_pool(name="small", bufs=1))
        sym = ctx.enter_context(tc.tile_pool(name="sym", bufs=1))
        symi = ctx.enter_context(tc.tile_pool(name="symi", bufs=2))
        entp = ctx.enter_context(tc.tile_pool(name="entp", bufs=2))
        ps_score = ctx.enter_context(tc.tile_pool(name="ps_score", bufs=2, space="PSUM"))
        ps_tr = ctx.enter_context(tc.tile_pool(name="ps_tr", bufs=1, space="PSUM"))
        ps_trb = ctx.enter_context(tc.tile_pool(name="ps_trb", bufs=2, space="PSUM"))
        ps_proj = ctx.enter_context(tc.tile_pool(name="ps_proj", bufs=1, space="PSUM"))
        early = ExitStack()
        early1 = early.enter_context(tc.tile_pool(name="early1", bufs=2))
        earlys = early.enter_context(tc.tile_pool(name="earlys", bufs=4))
        earlyb = early.enter_context(tc.tile_pool(name="earlyb", bufs=1))
        enpool = []   # created after early pools close

        # ---- constants
        ident_sb = const.tile([P, P], f32)
        nc.sync.dma_start(ident_sb[:], ident[:])
        identb_sb = const.tile([P, P], b16)
        nc.sync.dma_start(identb_sb[:], identb[:])
        ones128_sb = const.tile([P, 1], f32)
        nc.sync.dma_start(ones128_sb[:], ones128[:])
        padm_sb = const.tile([P, 1], f32)
        nc.sync.dma_start(padm_sb[:], padm[:])

        # ---- tail = head + pred -> tailT [128d, dk*32+r] bf16
        tail_f = small.tile([R, D], f32, tag="tailf")
        h_t = small.tile([R, D], f32, tag="hp")
        p_t = small.tile([R, D], f32, tag="hp2")
        nc.sync.dma_start(h_t[:], hemb[:])
        nc.sync.dma_start(p_t[:], pemb[:])
        nc.vector.tensor_tensor(out=tail_f[:], in0=h_t[:], in1=p_t[:], op=op.add)
        tailT = const.tile([P, 4 * R], b16)
        for dk in range(4):
            pt = ps_tr.tile([P, 512], f32, tag="tr")
            nc.tensor.transpose(out=pt[:, :R], in_=tail_f[:, dk * P:(dk + 1) * P],
                                identity=ident_sb[:R, :R])
            nc.vector.tensor_copy(out=tailT[:, dk * R:(dk + 1) * R], in_=pt[:, :R])

        # ---- symbolic stage 1: 4 x (dma hv slice + local_scatter)
        stg = []
        for g in range(NG):
            hvg = early1.tile([P, GR * T], b16, tag="hvg")
            nc.scalar.dma_start(hvg[:], hv_sb_d[:, g * GR * T:(g + 1) * GR * T])
            i1 = early1.tile([P, GR * T], i16, tag="idx1")
            nc.scalar.dma_start(i1[:], idx1_d[g])
            st = earlys.tile([P, C1 * P], b16, tag="stg")
            nc.gpsimd.local_scatter(st[:], hvg[:], i1[:], channels=P,
                                    num_elems=C1 * P, num_idxs=GR * T)
            stg.append(st)

        # ---- symbolic small input DMAs (scalar-engine DMA queue)
        idx2_t = earlyb.tile([P, BLEN], i16, tag="idx2")
        nc.scalar.dma_start(idx2_t[:], idx2_d[:])
        valM_t = sym.tile([P, NCH * MSTRIDE], b16, tag="valM")
        nc.scalar.dma_start(valM_t[:], valM_d[:])
        idx2d_t = sym.tile([P, NDUPE, C2 * P], i16, tag="idx2d")
        nc.scalar.dma_start(idx2d_t[:], idx2d_d[:].rearrange("k p n -> p k n"))
        valM2_t = sym.tile([P, NDUPE, NCH * M2CAP], b16, tag="valM2")
        nc.scalar.dma_start(valM2_t[:], valM2_d[:].rearrange("k p n -> p k n"))
        idx3_t = sym.tile([P, NCH * SING], i16, tag="idx3")
        nc.scalar.dma_start(idx3_t[:], idx3_d[:])
        idxsp_t = sym.tile([P, NSPILL, NCH * SPSTRIDE], i16, tag="idxsp")
        nc.scalar.dma_start(idxsp_t[:], idxsp_d[:].rearrange("k p n -> p k n"))

        Bt = earlyb.tile([P, BLEN], b16, tag="B")
        M_t = sym.tile([P, NCH * MSTRIDE], b16, tag="M")
        M2 = []
        spill = sym.tile([P, NCH, SPSTRIDE], b16, tag="spill")
        sym3 = big.tile([P, T, R], b16, tag="bigB")
        sym3f = sym3[:].rearrange("p t r -> p (t r)")

        def do_b_transposes(g):
            if True:
                for c in range(C1):
                    pt = ps_trb.tile([P, 512], b16, tag="trb")
                    nc.tensor.transpose(out=pt[:, :P],
                                        in_=stg[g][:, c * P:(c + 1) * P],
                                        identity=identb_sb[:])
                    nc.vector.tensor_copy(
                        out=Bt[:, (g * C1 + c) * P:(g * C1 + c + 1) * P],
                        in_=pt[:, :P])

        def do_m_scatter():
            nc.gpsimd.local_scatter(M_t[:], Bt[:], idx2_t[:], channels=P,
                                    num_elems=NCH * MSTRIDE, num_idxs=BLEN)

        def do_dupe_pass(k):
            idxd_t = symi.tile([P, NCH * MSTRIDE], i16, tag="idxd")
            nc.scalar.dma_start(idxd_t[:], idxd_d[k])
            st2 = symi.tile([P, C2 * P], b16, tag="stg2")
            nc.gpsimd.local_scatter(st2[:], M_t[:], idxd_t[:], channels=P,
                                    num_elems=C2 * P, num_idxs=NCH * MSTRIDE)
            Bd = symi.tile([P, C2 * P], b16, tag="Bd")
            for c in range(C2):
                pt = ps_trb.tile([P, 512], b16, tag="trb")
                nc.tensor.transpose(out=pt[:, :P], in_=st2[:, c * P:(c + 1) * P],
                                    identity=identb_sb[:])
                nc.vector.tensor_copy(out=Bd[:, c * P:(c + 1) * P], in_=pt[:, :P])
            m2t = sym.tile([P, NCH * M2CAP], b16, tag=f"M2_{k}")
            nc.gpsimd.local_scatter(m2t[:], Bd[:], idx2d_t[:, k, :], channels=P,
                                    num_elems=NCH * M2CAP, num_idxs=C2 * P)
            M2.append(m2t)

        def close_early():
            early.close()
            enp = ctx.enter_context(tc.tile_pool(name="enp", bufs=3))
            enpool.append(enp)

        def do_symbolic_finish():
            nc.vector.tensor_tensor(out=M_t[:], in0=M_t[:], in1=valM_t[:],
                                    op=op.mult)
            a = M_t[:]
            in0 = bass.AP(a.tensor, a.offset + SING,
                          [list(a.ap[0]), [MSTRIDE, NCH], [2, PAIR]])
            in1 = bass.AP(a.tensor, a.offset + SING + 1,
                          [list(a.ap[0]), [MSTRIDE, NCH], [2, PAIR]])
            nc.vector.tensor_tensor(out=spill[:, :, :PAIR], in0=in0, in1=in1,
                                    op=op.add)
            for k in range(NDUPE):
                m2v = M2[k][:].rearrange("p (ch s) -> p ch s", s=M2CAP)
                nc.vector.tensor_tensor(
                    out=m2v, in0=m2v,
                    in1=valM2_t[:, k, :].rearrange("p (ch s) -> p ch s", s=M2CAP),
                    op=op.mult)
                nc.vector.tensor_copy(
                    out=spill[:, :, PAIR + k * M2CAP:PAIR + (k + 1) * M2CAP],
                    in_=m2v)
            for c in range(NCH):
                ne = _chunk_elems(c)
                nc.gpsimd.local_scatter(
                    sym3f[:, c * TCH * R: c * TCH * R + ne],
                    M_t[:, c * MSTRIDE:c * MSTRIDE + SING],
                    idx3_t[:, c * SING:(c + 1) * SING],
                    channels=P, num_elems=ne, num_idxs=SING)
            for (L, c) in _SPILL_PLAN:
                ne = _chunk_elems(c)
                ssp = sym.tile([P, TCH * R], b16, tag="ssp")
                nc.gpsimd.local_scatter(
                    ssp[:, :ne], spill[:, c, :],
                    idxsp_t[:, L, c * SPSTRIDE:(c + 1) * SPSTRIDE],
                    channels=P, num_elems=ne, num_idxs=SPSTRIDE)
                nc.vector.tensor_tensor(
                    out=sym3f[:, c * TCH * R: c * TCH * R + ne],
                    in0=sym3f[:, c * TCH * R: c * TCH * R + ne],
                    in1=ssp[:, :ne], op=op.add)

        def bcast8(ps_tile, r0, cnt8, nr):
            a = ps_tile[:, r0:r0 + nr]
            return bass.AP(a.tensor, a.offset,
                           [list(a.ap[0]), [0, cnt8], list(a.ap[1])])

        def bcast_full(ps_tile, nr):
            a = ps_tile[:, :nr]
            return bass.AP(a.tensor, a.offset,
                           [list(a.ap[0]), [0, T], list(a.ap[1])])

        sy_flat = sym3[:].rearrange("p t r -> p (t r)")
        sy_rt = sym3[:].rearrange("p t r -> p r t")

        y1 = small.tile([P, R], f32, tag="y1")

        def do_y1():
            nc.vector.tensor_reduce(out=y1[:], in_=sy_rt,
                                    axis=mybir.AxisListType.X, op=op.add)

        # ---- helpers
        def col_reduce_bcast(x_rc, nr, op_red, post, tagp):
            pt = ps_tr.tile([P, 512], f32, tag="tr")
            nc.tensor.transpose(out=pt[:nr, :P], in_=x_rc, identity=ident_sb[:])
            v = small.tile([nr, 1], f32, tag="v" + tagp)
            nc.vector.tensor_reduce(out=v[:], in_=pt[:nr, :P],
                                    axis=mybir.AxisListType.X, op=op_red)
            v2 = small.tile([nr, 1], b16, tag="w" + tagp)
            post(v, v2)
            pb = ps_trb.tile([P, 512], b16, tag="trb")
            vb = v2[:].to_broadcast([nr, P])
            nc.tensor.transpose(out=pb[:, :nr], in_=vb, identity=identb_sb[:nr, :nr])
            return pb

        def recip_post(v, v2):
            nc.vector.reciprocal(out=v2[:], in_=v[:])

        def clipmax_recip(v, v2):
            nc.vector.tensor_scalar(out=v[:], in0=v[:], scalar1=float(CLIP),
                                    scalar2=None, op0=op.max)
            nc.vector.reciprocal(out=v2[:], in_=v[:])

        def do_prescale_sym():
            yB = col_reduce_bcast(y1[:], R, op.add, clipmax_recip, "y")
            yBs.append(yB)
            nc.vector.tensor_tensor(out=sy_flat, in0=sy_flat,
                                    in1=bcast_full(yB, R), op=op.mult)

        hooks = {5: lambda: do_b_transposes(0),
                 7: lambda: do_b_transposes(1),
                 9: lambda: do_b_transposes(2),
                 11: lambda: do_b_transposes(3),
                 13: do_m_scatter,
                 14: close_early,
                 15: lambda: do_dupe_pass(0),
                 17: lambda: do_dupe_pass(1),
                 19: lambda: do_dupe_pass(2),
                 21: do_symbolic_finish,
                 26: do_y1,
                 28: do_prescale_sym}
        yBs = []

        # ---- score: per group: matmuls -> copy -> (mask) -> exp -> s1 partial
        score3 = big.tile([P, T, R], b16, tag="bigA")
        s1p = small.tile([P, NGRP, R], f32, tag="s1p")
        for grp in range(NGRP):
            if grp in hooks:
                hooks[grp]()
            t0 = grp * NWIN
            cnt = min(NWIN, T - t0)
            et = entp.tile([P, NWIN, 4, P], b16, tag="et")
            nc.sync.dma_start(et[:], entT[grp])
            pss = ps_score.tile([P, TPAD], f32, tag="ps_s")
            for k in range(cnt):
                for dk in range(4):
                    nc.tensor.matmul(out=pss[:, 32 * k:32 * k + 32],
                                     lhsT=et[:, k, dk, :],
                                     rhs=tailT[:, dk * R:(dk + 1) * R],
                                     start=(dk == 0), stop=(dk == 3))
            nc.vector.tensor_copy(out=score3[:, t0:t0 + cnt, :],
                                  in_=pss[:, :32 * cnt])
            if grp == NGRP - 1:
                nc.vector.tensor_scalar(out=score3[:, T - 1:T, :],
                                        in0=score3[:, T - 1:T, :],
                                        scalar1=padm_sb[:], scalar2=None,
                                        op0=op.add)
            sl_flat = score3[:, t0:t0 + cnt, :].rearrange("p t r -> p (t r)")
            nc.scalar.activation(out=sl_flat, in_=sl_flat, func=Exp)
            nc.vector.tensor_reduce(
                out=s1p[:, grp, :],
                in_=score3[:, t0:t0 + cnt, :].rearrange("p t r -> p r t"),
                axis=mybir.AxisListType.X, op=op.add)

        # ---- normalizers: sB = 1/sum(exp), yB = 1/max(CLIP, sum(sym))
        s1 = small.tile([P, R], f32, tag="s1")
        nc.vector.tensor_reduce(out=s1[:], in_=s1p[:].rearrange("p g r -> p r g"),
                                axis=mybir.AxisListType.X, op=op.add)

        sB = col_reduce_bcast(s1[:], R, op.add, recip_post, "s")
        sc_flat = score3[:].rearrange("p t r -> p (t r)")
        nc.vector.tensor_tensor(out=sc_flat, in0=sc_flat, in1=bcast_full(sB, R),
                                op=op.mult)

        # ---- agg_b = (sym0'+e0')*(sym1'+e1') in 6 big chunks
        agg_b = big.tile([P, T, BLOC], b16, tag="aggb")
        ACH = 66
        ft1 = small.tile([P, ACH, BLOC], b16, tag="ft1")
        ft2 = small.tile([P, ACH, BLOC], b16, tag="ft2")
        for c0 in range(0, T, ACH):
            cc = min(ACH, T - c0)
            sl = slice(c0, c0 + cc)
            nc.vector.tensor_tensor(out=ft1[:, :cc], in0=sym3[:, sl, :BLOC],
                                    in1=score3[:, sl, :BLOC], op=op.add)
            nc.vector.tensor_tensor(out=ft2[:, :cc], in0=sym3[:, sl, BLOC:],
                                    in1=score3[:, sl, BLOC:], op=op.add)
            nc.vector.tensor_tensor(out=agg_b[:, sl, :], in0=ft1[:, :cc],
                                    in1=ft2[:, :cc], op=op.mult)

        # ---- projection loop: pure DMA + matmul
        pso = ps_proj.tile([BLOC, D], f32)
        enp = enpool[0]
        for g in range(NPGRP):
            nt0 = 16 * g
            cnt16 = min(16, T - nt0)
            en = enp.tile([P, 16, D], b16, tag="en")
            nc.sync.dma_start(en[:], entN[g])
            for k in range(cnt16):
                nt = nt0 + k
                nc.tensor.matmul(out=pso[:], lhsT=agg_b[:, nt, :],
                                 rhs=en[:, k, :],
                                 start=(nt == 0), stop=(nt == T - 1))

        # ---- denominator: sum over n of agg_unnorm, then out = pso / denom
        g1 = small.tile([P, BLOC], f32, tag="g1")
        nc.vector.tensor_reduce(out=g1[:],
                                in_=agg_b[:].rearrange("p t r -> p r t"),
                                axis=mybir.AxisListType.X, op=op.add)
        pd = ps_tr.tile([P, 512], f32, tag="tr")
        nc.tensor.matmul(out=pd[:BLOC, :1], lhsT=g1[:], rhs=ones128_sb[:],
                         start=True, stop=True)
        dclip = small.tile([BLOC, 1], f32, tag="dclip")
        nc.vector.tensor_scalar(out=dclip[:], in0=pd[:BLOC, :1],
                                scalar1=float(CLIP), scalar2=None, op0=op.max)
        drcp = small.tile([BLOC, 1], f32, tag="drcp")
        nc.vector.reciprocal(out=drcp[:], in_=dclip[:])
        out_sb = small.tile([BLOC, D], f32, tag="outsb")
        nc.vector.tensor_scalar(out=out_sb[:], in0=pso[:], scalar1=drcp[:],
                                scalar2=None, op0=op.mult)
        nc.sync.dma_start(out[:], out_sb[:])

    nc.compile()
    return nc


_PROGRAM = None


def kernel(entity_embedding, head_vector, head_emb, pred_emb,
           edge_val, edge_src, edge_dst):
    global _PROGRAM
    from concourse.bass_utils import run_bass_kernel_spmd

    in_maps = _build_host_inputs(entity_embedding, head_vector,
                                 head_emb, pred_emb,
                                 edge_val, edge_src, edge_dst)
    if _PROGRAM is None:
        _PROGRAM = build_program()
    res = run_bass_kernel_spmd(_PROGRAM, in_maps, list(range(NCORES)))
    out = np.empty((B, D), np.float32)
    for c in range(NCORES):
        out[c * BLOC:(c + 1) * BLOC] = res.results[c]["out"]
    return out


if __name__ == "__main__":
    import reference
    inputs = {k: np.asarray(v) for k, v in reference.setup_inputs().items()}
    got = kernel(**inputs)
    want = np.asarray(reference.reference(**inputs))
    err = np.abs(got - want).max() / np.abs(want).max()
    print("Relative error:", err)


# revision 11
# speedup vs baseline: 4.6437x; 1.2178x over previous
"""Trainium2 Bass kernel for NeuralSymbolicMP layer (gnn_message_passing).

Batch-sharded over B across 8 NeuronCores; each core handles 32 (atomic,
batch) rows x all N entities.

Symbolic message passing runs as a GpSimd local_scatter permutation network:
  hv (src-partition layout) --local_scatter--> staging --PE transpose-->
  B (dst-partition layout) --local_scatter--> M (per-edge, dst-ordered)
  --x val--> --local_scatter--> dense sym3 [128, T, R]
with duplicate-src edges resolved by sibling-read passes over M and
duplicate-dst edges by pair slots + spill layers.

Neural side (score matmul, softmax, clip-norm chain, projection) is a dense
PE/DVE pipeline over entity tiles streamed from HBM in bf16.
"""

import numpy as np
import ml_dtypes

A, B, N, D, E = 2, 128, 50000, 512, 4096
CLIP = 1e-14

NCORES = 8
BLOC = B // NCORES          # 16 batch per core
R = A * BLOC                # 32 rows per core
P = 128
T = 391                     # n tiles: 391*128 = 50048
NPAD = T * P                # 50048
NWIN = 13                   # score psum windows of 32 tiles
TPAD = NWIN * 32            # 416

# symbolic permutation network constants
NG = 4                      # row groups for staging
GR = R // NG                # 8 rows per group
C1 = 13                     # staging chunks per group
TCH = 56                    # t-tiles per M chunk
NCH = 7                     # M chunks
SING = 224                  # singles per (p, chunk)
PAIR = 16                   # pair slots per (p, chunk)
MSTRIDE = SING + 2 * PAIR   # 256
C2 = 6                      # dupe staging chunks
M2CAP = 24                  # M2 capacity per (p, chunk)
NDUPE = 3                   # dupe passes
SPSTRIDE = PAIR + NDUPE * M2CAP  # 88
NSPILL = 3                  # spill layers
SYM_LEN = T * R             # 12512
BLEN = NG * C1 * P          # 6656

bf16 = ml_dtypes.bfloat16

# spill build plan: set of (layer, chunk) with any entry on any core.
# populated by _build_host_inputs before build_program is called.
_SPILL_PLAN = [(L, c) for L in range(NSPILL) for c in range(NCH)]


def _chunk_elems(ch):
    t0, t1 = ch * TCH, min((ch + 1) * TCH, T)
    return (t1 - t0) * R


def _cumcount(keys, sel):
    """For each selected element, its 0-based rank among same-key elements."""
    idxs = np.nonzero(sel)[0]
    if idxs.size == 0:
        return idxs, np.zeros(0, np.int64)
    kk = keys[idxs]
    o = np.argsort(kk, kind="stable")
    k = kk[o]
    n = np.ones(idxs.size, bool)
    n[1:] = k[1:] != k[:-1]
    g = np.cumsum(n) - 1
    f = np.zeros(g.max() + 1, np.int64)
    f[g[n]] = np.nonzero(n)[0]
    cnt_sorted = np.arange(idxs.size) - f[g]
    out = np.empty(idxs.size, np.int64)
    out[o] = cnt_sorted
    return idxs, out


def _prep_symbolic(hv_core, src, dst, val):
    """hv_core [R, NPAD] f32, src/dst/val [R, E]. Returns device arrays +
    spill plan entries."""
    r = np.repeat(np.arange(R), E)
    s = src.ravel().astype(np.int64)
    d = dst.ravel().astype(np.int64)
    v = val.ravel().astype(np.float32)
    NE = r.size

    p_s, t_s = s % P, s // P
    p_d, t_d = d % P, d // P
    g = r // GR
    ch = t_d // TCH

    # src-group ranks
    key_s = r * NPAD + s
    o = np.argsort(key_s, kind="stable")
    ks = key_s[o]
    newg = np.ones(NE, bool)
    newg[1:] = ks[1:] != ks[:-1]
    gid = np.cumsum(newg) - 1
    first = np.zeros(gid.max() + 1, np.int64)
    first[gid[newg]] = np.nonzero(newg)[0]
    srank = np.empty(NE, np.int64)
    srank[o] = np.arange(NE) - first[gid]
    assert srank.max() <= NDUPE, f"src dupe depth {srank.max()}"
    main = srank == 0
    sibling = np.empty(NE, np.int64)
    sibling[o] = o[first[gid]]

    # staging chunks for main edges
    mi, c1 = _cumcount((g * P + p_s) * P + p_d, main)
    assert c1.max() < C1, f"C1 overflow {c1.max() + 1}"

    # dst groups: mains first
    key_d = r * NPAD + d
    od = np.argsort(key_d * 8 + np.minimum(srank, 7), kind="stable")
    kd = key_d[od]
    ngd = np.ones(NE, bool)
    ngd[1:] = kd[1:] != kd[:-1]
    gidd = np.cumsum(ngd) - 1
    fd = np.zeros(gidd.max() + 1, np.int64)
    fd[gidd[ngd]] = np.nonzero(ngd)[0]
    drank = np.empty(NE, np.int64)
    drank[od] = np.arange(NE) - fd[gidd]
    gmain = np.zeros(gidd.max() + 1, np.int64)
    np.add.at(gmain, gidd, main[od].astype(np.int64))
    qmain = np.empty(NE, np.int64)
    qmain[od] = gmain[gidd]

    is_single = main & (qmain == 1)
    is_pairm = main & (qmain >= 2)
    cellM = p_d * NCH + ch

    si, scnt = _cumcount(cellM, is_single)
    assert scnt.size == 0 or scnt.max() < SING, f"SING overflow {scnt.max()+1}"
    pair_lead = is_pairm & (drank % 2 == 0)
    pi, pcnt = _cumcount(cellM, pair_lead)
    assert pcnt.size == 0 or pcnt.max() < PAIR, f"PAIR overflow {pcnt.max()+1}"
    pairslot = {(key_d[e], drank[e] // 2): c for e, c in zip(pi, pcnt)}

    Mslot = np.full(NE, -1, np.int64)
    Mslot[si] = ch[si] * MSTRIDE + scnt
    pm = np.nonzero(is_pairm)[0]
    for e in pm:
        c = pairslot[(key_d[e], drank[e] // 2)]
        Mslot[e] = ch[e] * MSTRIDE + SING + 2 * c + (drank[e] % 2)

    m2slot = np.full(NE, -1, np.int64)
    m2pass = np.full(NE, -1, np.int64)
    for k in range(NDUPE):
        ki, kcnt = _cumcount(cellM, srank == k + 1)
        assert kcnt.size == 0 or kcnt.max() < M2CAP, f"M2 overflow {kcnt.max()+1}"
        m2slot[ki] = ch[ki] * M2CAP + kcnt
        m2pass[ki] = k

    c2 = np.full(NE, -1, np.int64)
    for k in range(NDUPE):
        ki, cko = _cumcount(p_d[sibling] * P + p_d, srank == k + 1)
        assert cko.size == 0 or cko.max() < C2, f"C2 overflow {cko.max()+1}"
        c2[ki] = cko

    # spill layers
    sym_of = t_d * R + r
    spill_ctr = {}
    pair_spill = {}
    for e in pi:
        kk = (p_d[e], sym_of[e])
        L = spill_ctr.get(kk, 0)
        assert L < NSPILL, "spill overflow"
        spill_ctr[kk] = L + 1
        pair_spill[(key_d[e], drank[e] // 2)] = L
    m2_spill = np.full(NE, -1, np.int64)
    for e in np.nonzero(srank >= 1)[0]:
        kk = (p_d[e], sym_of[e])
        L = spill_ctr.get(kk, 0)
        assert L < NSPILL, "spill overflow"
        spill_ctr[kk] = L + 1
        m2_spill[e] = L

    # ---- device arrays
    arr = {}
    idx1 = np.full((NG, P, GR * T), -1, np.int16)
    ma = np.nonzero(main)[0]
    idx1[g[ma], p_s[ma], (r[ma] % GR) * T + t_s[ma]] = \
        (c1 * P + p_d[mi]).astype(np.int16)
    arr["idx1"] = idx1

    idx2 = np.full((P, BLEN), -1, np.int16)
    idx2[p_d[mi], (g[mi] * C1 + c1) * P + p_s[mi]] = Mslot[mi].astype(np.int16)
    arr["idx2"] = idx2

    valM = np.zeros((P, NCH * MSTRIDE), np.float32)
    valM[p_d[mi], Mslot[mi]] = v[mi]
    arr["valM"] = valM.astype(bf16)

    idxd = np.full((NDUPE, P, NCH * MSTRIDE), -1, np.int16)
    idx2d = np.full((NDUPE, P, C2 * P), -1, np.int16)
    valM2 = np.zeros((NDUPE, P, NCH * M2CAP), np.float32)
    for k in range(NDUPE):
        ki = np.nonzero(srank == k + 1)[0]
        sib = sibling[ki]
        idxd[k, p_d[sib], Mslot[sib]] = (c2[ki] * P + p_d[ki]).astype(np.int16)
        idx2d[k, p_d[ki], c2[ki] * P + p_d[sib]] = m2slot[ki].astype(np.int16)
        valM2[k, p_d[ki], m2slot[ki]] = v[ki]
    arr["idxd"] = idxd
    arr["idx2d"] = idx2d
    arr["valM2"] = valM2.astype(bf16)

    idx3 = np.full((P, NCH * SING), -1, np.int16)
    idx3[p_d[si], ch[si] * SING + scnt] = \
        ((t_d[si] - ch[si] * TCH) * R + r[si]).astype(np.int16)
    arr["idx3"] = idx3

    idxsp = np.full((NSPILL, P, NCH * SPSTRIDE), -1, np.int16)
    plan = set()
    for e in pi:
        c = pairslot[(key_d[e], drank[e] // 2)]
        L = pair_spill[(key_d[e], drank[e] // 2)]
        idxsp[L, p_d[e], ch[e] * SPSTRIDE + c] = \
            (t_d[e] - ch[e] * TCH) * R + r[e]
        plan.add((L, int(ch[e])))
    for e in np.nonzero(srank >= 1)[0]:
        k = m2pass[e]
        L = m2_spill[e]
        rel = m2slot[e] - ch[e] * M2CAP
        idxsp[L, p_d[e], ch[e] * SPSTRIDE + PAIR + k * M2CAP + rel] = \
            (t_d[e] - ch[e] * TCH) * R + r[e]
        plan.add((int(L), int(ch[e])))
    arr["idxsp"] = idxsp

    arr["hv_sb"] = np.ascontiguousarray(
        hv_core.reshape(R, T, P).transpose(2, 0, 1).reshape(P, R * T)
    ).astype(bf16)
    return arr, plan


def _prep_core(core, head_vector, head_emb, pred_emb, edge_val, edge_src,
               edge_dst):
    b0 = core * BLOC
    hv = np.zeros((R, NPAD), np.float32)
    ev = np.empty((R, E), np.float32)
    es = np.empty((R, E), np.int64)
    ed = np.empty((R, E), np.int64)
    hemb = np.empty((R, D), np.float32)
    pemb = np.empty((R, D), np.float32)
    for a in range(A):
        sl = slice(a * BLOC, (a + 1) * BLOC)
        hv[sl, :N] = head_vector[a, b0:b0 + BLOC]
        ev[sl] = edge_val[a, b0:b0 + BLOC]
        es[sl] = edge_src[a, b0:b0 + BLOC]
        ed[sl] = edge_dst[a, b0:b0 + BLOC]
        hemb[sl] = head_emb[a, b0:b0 + BLOC]
        pemb[sl] = pred_emb[a, b0:b0 + BLOC]

    arr, plan = _prep_symbolic(hv, es, ed, ev)
    arr["hemb"] = hemb
    arr["pemb"] = pemb
    return arr, plan


def _build_host_inputs(entity_embedding, head_vector, head_emb, pred_emb,
                       edge_val, edge_src, edge_dst):
    global _SPILL_PLAN
    entity_pad = np.zeros((NPAD, D), np.float32)
    entity_pad[:N] = entity_embedding
    # proj moving operand entN: [g, p, k*512+dd] = entity[(8g+k)*128+p, dd]
    npadg = ((T + 15) // 16) * 16 * P
    en_t = np.zeros((npadg // P, P, D), np.float32)
    en_t[:T] = entity_pad.reshape(T, P, D)
    entN = np.ascontiguousarray(
        en_t.reshape(-1, 16, P, D).transpose(0, 2, 1, 3).reshape(-1, P, 16 * D)
    ).astype(bf16)
    # score stationary entT: [g, p, k*512+dk*128+j] = entity[(13g+k)*128+j, dk*128+p]
    ngrp = (T + NWIN - 1) // NWIN
    et_t = np.zeros((ngrp * NWIN, P, D), np.float32)
    et_t[:T] = entity_pad.reshape(T, P, D)
    et5 = et_t.reshape(ngrp, NWIN, P, 4, P).transpose(0, 4, 1, 3, 2)
    entT = np.ascontiguousarray(et5.reshape(ngrp, P, NWIN * D)).astype(bf16)
    ident = np.eye(P, dtype=np.float32)
    identb = np.eye(P, dtype=bf16)
    ones128 = np.ones((P, 1), np.float32)
    padm = np.zeros((P, 1), np.float32)
    padm[N - (T - 1) * P:] = -1e30

    in_maps = []
    plan_all = set()
    for core in range(NCORES):
        m, plan = _prep_core(core, head_vector, head_emb, pred_emb,
                             edge_val, edge_src, edge_dst)
        plan_all |= plan
        m["entT"] = entT
        m["entN"] = entN
        m["ident"] = ident
        m["identb"] = identb
        m["ones128"] = ones128
        m["padm"] = padm
        in_maps.append(m)
    _SPILL_PLAN = sorted(plan_all, key=lambda lc: (lc[1], lc[0]))
    return in_maps


# ---------------------------------------------------------------------------
# Bass program
# ---------------------------------------------------------------------------

def build_program():
    from contextlib import ExitStack
    import concourse.bass as bass
    import concourse.tile as tile
    from concourse import bacc, mybir
    from concourse.alu_op_type import AluOpType as op
    import bass_rust

    dt = mybir.dt
    f32, b16, i16 = dt.float32, dt.bfloat16, dt.int16
    Exp = bass_rust.ActivationFunctionType.Exp

    nc = bacc.Bacc("TRN2", target_bir_lowering=False, debug=False,
                   num_devices=NCORES)

    def din(name, shape, dtype):
        return nc.dram_tensor(name, list(shape), dtype, kind="ExternalInput").ap()

    NGRP = (T + NWIN - 1) // NWIN
    NPGRP = (T + 15) // 16
    entT = din("entT", (NGRP, P, NWIN * D), b16)
    entN = din("entN", (NPGRP, P, 16 * D), b16)
    hemb = din("hemb", (R, D), f32)
    pemb = din("pemb", (R, D), f32)
    ident = din("ident", (P, P), f32)
    identb = din("identb", (P, P), b16)
    ones128 = din("ones128", (P, 1), f32)
    padm = din("padm", (P, 1), f32)
    hv_sb_d = din("hv_sb", (P, R * T), b16)
    idx1_d = din("idx1", (NG, P, GR * T), i16)
    idx2_d = din("idx2", (P, BLEN), i16)
    valM_d = din("valM", (P, NCH * MSTRIDE), b16)
    idxd_d = din("idxd", (NDUPE, P, NCH * MSTRIDE), i16)
    idx2d_d = din("idx2d", (NDUPE, P, C2 * P), i16)
    valM2_d = din("valM2", (NDUPE, P, NCH * M2CAP), b16)
    idx3_d = din("idx3", (P, NCH * SING), i16)
    idxsp_d = din("idxsp", (NSPILL, P, NCH * SPSTRIDE), i16)

    out = nc.dram_tensor("out", [BLOC, D], f32, kind="ExternalOutput").ap()

    with tile.TileContext(nc) as tc, ExitStack() as ctx:
        ctx.enter_context(nc.allow_low_precision(
            reason="bf16 storage is deliberate; reductions accumulate f32"))
        const = ctx.enter_context(tc.tile_pool(name="const", bufs=1))
        big = ctx.enter_context(tc.tile_pool(name="big", bufs=1))
        small = ctx.enter_context(tc.tile_pool(name="small", bufs=1))
        sym = ctx.enter_context(tc.tile_pool(name="sym", bufs=1))
        symi = ctx.enter_context(tc.tile_pool(name="symi", bufs=2))
        entp = ctx.enter_context(tc.tile_pool(name="entp", bufs=2))
        ps_score = ctx.enter_context(tc.tile_pool(name="ps_score", bufs=2, space="PSUM"))
        ps_tr = ctx.enter_context(tc.tile_pool(name="ps_tr", bufs=1, space="PSUM"))
        ps_trb = ctx.enter_context(tc.tile_pool(name="ps_trb", bufs=2, space="PSUM"))
        ps_proj = ctx.enter_context(tc.tile_pool(name="ps_proj", bufs=1, space="PSUM"))
        early = ExitStack()
        early1 = early.enter_context(tc.tile_pool(name="early1", bufs=2))
        earlys = early.enter_context(tc.tile_pool(name="earlys", bufs=4))
        earlyb = early.enter_context(tc.tile_pool(name="earlyb", bufs=1))
        enpool = []   # created after early pools close

        # ---- constants
        ident_sb = const.tile([P, P], f32)
        nc.sync.dma_start(ident_sb[:], ident[:])
        identb_sb = const.tile([P, P], b16)
        nc.sync.dma_start(identb_sb[:], identb[:])
        ones128_sb = const.tile([P, 1], f32)
        nc.sync.dma_start(ones128_sb[:], ones128[:])
        padm_sb = const.tile([P, 1], f32)
        nc.sync.dma_start(padm_sb[:], padm[:])

        # ---- tail = head + pred -> tailT [128d, dk*32+r] bf16
        tail_f = small.tile([R, D], f32, tag="tailf")
        h_t = small.tile([R, D], f32, tag="hp")
        p_t = small.tile([R, D], f32, tag="hp2")
        nc.sync.dma_start(h_t[:], hemb[:])
        nc.sync.dma_start(p_t[:], pemb[:])
        nc.vector.tensor_tensor(out=tail_f[:], in0=h_t[:], in1=p_t[:], op=op.add)
        tailT = const.tile([P, 4 * R], b16)
        for dk in range(4):
            pt = ps_tr.tile([P, 512], f32, tag="tr")
            nc.tensor.transpose(out=pt[:, :R], in_=tail_f[:, dk * P:(dk + 1) * P],
                                identity=ident_sb[:R, :R])
            nc.vector.tensor_copy(out=tailT[:, dk * R:(dk + 1) * R], in_=pt[:, :R])

        # ---- symbolic stage 1: 4 x (dma hv slice + local_scatter)
        stg = []
        for g in range(NG):
            hvg = early1.tile([P, GR * T], b16, tag="hvg")
            nc.scalar.dma_start(hvg[:], hv_sb_d[:, g * GR * T:(g + 1) * GR * T])
            i1 = early1.tile([P, GR * T], i16, tag="idx1")
            nc.scalar.dma_start(i1[:], idx1_d[g])
            st = earlys.tile([P, C1 * P], b16, tag="stg")
            nc.gpsimd.local_scatter(st[:], hvg[:], i1[:], channels=P,
                                    num_elems=C1 * P, num_idxs=GR * T)
            stg.append(st)

        # ---- symbolic small input DMAs (scalar-engine DMA queue)
        idx2_t = earlyb.tile([P, BLEN], i16, tag="idx2")
        nc.scalar.dma_start(idx2_t[:], idx2_d[:])
        valM_t = sym.tile([P, NCH * MSTRIDE], b16, tag="valM")
        nc.scalar.dma_start(valM_t[:], valM_d[:])
        idx2d_t = sym.tile([P, NDUPE, C2 * P], i16, tag="idx2d")
        nc.scalar.dma_start(idx2d_t[:], idx2d_d[:].rearrange("k p n -> p k n"))
        valM2_t = sym.tile([P, NDUPE, NCH * M2CAP], b16, tag="valM2")
        nc.scalar.dma_start(valM2_t[:], valM2_d[:].rearrange("k p n -> p k n"))
        idx3_t = sym.tile([P, NCH * SING], i16, tag="idx3")
        nc.scalar.dma_start(idx3_t[:], idx3_d[:])
        idxsp_t = sym.tile([P, NSPILL, NCH * SPSTRIDE], i16, tag="idxsp")
        nc.scalar.dma_start(idxsp_t[:], idxsp_d[:].rearrange("k p n -> p k n"))

        Bt = earlyb.tile([P, BLEN], b16, tag="B")
        M_t = sym.tile([P, NCH * MSTRIDE], b16, tag="M")
        M2 = []
        spill = sym.tile([P, NCH, SPSTRIDE], b16, tag="spill")
        sym3 = big.tile([P, T, R], b16, tag="bigB")
        sym3f = sym3[:].rearrange("p t r -> p (t r)")

        def do_b_transposes(g):
            if True:
                for c in range(C1):
                    pt = ps_trb.tile([P, 512], b16, tag="trb")
                    nc.tensor.transpose(out=pt[:, :P],
                                        in_=stg[g][:, c * P:(c + 1) * P],
                                        identity=identb_sb[:])
                    nc.vector.tensor_copy(
                        out=Bt[:, (g * C1 + c) * P:(g * C1 + c + 1) * P],
                        in_=pt[:, :P])

        def do_m_scatter():
            nc.gpsimd.local_scatter(M_t[:], Bt[:], idx2_t[:], channels=P,
                                    num_elems=NCH * MSTRIDE, num_idxs=BLEN)

        def do_dupe_pass(k):
            idxd_t = symi.tile([P, NCH * MSTRIDE], i16, tag="idxd")
            nc.scalar.dma_start(idxd_t[:], idxd_d[k])
            st2 = symi.tile([P, C2 * P], b16, tag="stg2")
            nc.gpsimd.local_scatter(st2[:], M_t[:], idxd_t[:], channels=P,
                                    num_elems=C2 * P, num_idxs=NCH * MSTRIDE)
            Bd = symi.tile([P, C2 * P], b16, tag="Bd")
            for c in range(C2):
                pt = ps_trb.tile([P, 512], b16, tag="trb")
                nc.tensor.transpose(out=pt[:, :P], in_=st2[:, c * P:(c + 1) * P],
                                    identity=identb_sb[:])
                nc.vector.tensor_copy(out=Bd[:, c * P:(c + 1) * P], in_=pt[:, :P])
            m2t = sym.tile([P, NCH * M2CAP], b16, tag=f"M2_{k}")
            nc.gpsimd.local_scatter(m2t[:], Bd[:], idx2d_t[:, k, :], channels=P,
                                    num_elems=NCH * M2CAP, num_idxs=C2 * P)
            M2.append(m2t)

        def close_early():
            early.close()
            enp = ctx.enter_context(tc.tile_pool(name="enp", bufs=3))
            enpool.append(enp)

        def do_symbolic_finish():
            nc.vector.tensor_tensor(out=M_t[:], in0=M_t[:], in1=valM_t[:],
                                    op=op.mult)
            a = M_t[:]
            in0 = bass.AP(a.tensor, a.offset + SING,
                          [list(a.ap[0]), [MSTRIDE, NCH], [2, PAIR]])
            in1 = bass.AP(a.tensor, a.offset + SING + 1,
                          [list(a.ap[0]), [MSTRIDE, NCH], [2, PAIR]])
            nc.vector.tensor_tensor(out=spill[:, :, :PAIR], in0=in0, in1=in1,
                                    op=op.add)
            for k in range(NDUPE):
                m2v = M2[k][:].rearrange("p (ch s) -> p ch s", s=M2CAP)
                nc.vector.tensor_tensor(
                    out=m2v, in0=m2v,
                    in1=valM2_t[:, k, :].rearrange("p (ch s) -> p ch s", s=M2CAP),
                    op=op.mult)
                nc.vector.tensor_copy(
                    out=spill[:, :, PAIR + k * M2CAP:PAIR + (k + 1) * M2CAP],
                    in_=m2v)
            for c in range(NCH):
                ne = _chunk_elems(c)
                nc.gpsimd.local_scatter(
                    sym3f[:, c * TCH * R: c * TCH * R + ne],
                    M_t[:, c * MSTRIDE:c * MSTRIDE + SING],
                    idx3_t[:, c * SING:(c + 1) * SING],
                    channels=P, num_elems=ne, num_idxs=SING)
            for (L, c) in _SPILL_PLAN:
                ne = _chunk_elems(c)
                ssp = sym.tile([P, TCH * R], b16, tag="ssp")
                nc.gpsimd.local_scatter(
                    ssp[:, :ne], spill[:, c, :],
                    idxsp_t[:, L, c * SPSTRIDE:(c + 1) * SPSTRIDE],
                    channels=P, num_elems=ne, num_idxs=SPSTRIDE)
                nc.vector.tensor_tensor(
                    out=sym3f[:, c * TCH * R: c * TCH * R + ne],
                    in0=sym3f[:, c * TCH * R: c * TCH * R + ne],
                    in1=ssp[:, :ne], op=op.add)

        def bcast8(ps_tile, r0, cnt8, nr):
            a = ps_tile[:, r0:r0 + nr]
            return bass.AP(a.tensor, a.offset,
                           [list(a.ap[0]), [0, cnt8], list(a.ap[1])])

        def bcast_full(ps_tile, nr):
            a = ps_tile[:, :nr]
            return bass.AP(a.tensor, a.offset,
                           [list(a.ap[0]), [0, T], list(a.ap[1])])

        sy_flat = sym3[:].rearrange("p t r -> p (t r)")
        sy_rt = sym3[:].rearrange("p t r -> p r t")

        y1 = small.tile([P, R], f32, tag="y1")
        YQ = 98
        y1q = small.tile([P, 4, R], f32, tag="y1q")

        def do_y1q(q):
            t0q = q * YQ
            t1q = min((q + 1) * YQ, T)
            nc.vector.tensor_reduce(
                out=y1q[:, q, :],
                in_=sym3[:, t0q:t1q, :].rearrange("p t r -> p r t"),
                axis=mybir.AxisListType.X, op=op.add)

        # ---- helpers
        def col_reduce_bcast(x_rc, nr, op_red, post, tagp):
            pt = ps_tr.tile([P, 512], f32, tag="tr")
            nc.tensor.transpose(out=pt[:nr, :P], in_=x_rc, identity=ident_sb[:])
            v = small.tile([nr, 1], f32, tag="v" + tagp)
            nc.vector.tensor_reduce(out=v[:], in_=pt[:nr, :P],
                                    axis=mybir.AxisListType.X, op=op_red)
            v2 = small.tile([nr, 1], b16, tag="w" + tagp)
            post(v, v2)
            pb = ps_trb.tile([P, 512], b16, tag="trb")
            vb = v2[:].to_broadcast([nr, P])
            nc.tensor.transpose(out=pb[:, :nr], in_=vb, identity=identb_sb[:nr, :nr])
            return pb

        def recip_post(v, v2):
            nc.vector.reciprocal(out=v2[:], in_=v[:])

        def clipmax_recip(v, v2):
            nc.vector.tensor_scalar(out=v[:], in0=v[:], scalar1=float(CLIP),
                                    scalar2=None, op0=op.max)
            nc.vector.reciprocal(out=v2[:], in_=v[:])

        hooks = {5: lambda: do_b_transposes(0),
                 7: lambda: do_b_transposes(1),
                 9: lambda: do_b_transposes(2),
                 11: lambda: do_b_transposes(3),
                 13: do_m_scatter,
                 19: lambda: do_dupe_pass(0),
                 21: lambda: do_dupe_pass(1),
                 23: lambda: do_dupe_pass(2),
                 24: do_symbolic_finish,
                 27: lambda: do_y1q(0),
                 28: lambda: do_y1q(1),
                 29: lambda: do_y1q(2),
                 30: close_early}

        # ---- score: per group: matmuls -> copy -> (mask) -> exp -> s1 partial
        score3 = big.tile([P, T, R], b16, tag="bigA")
        s1p = small.tile([P, NGRP, R], f32, tag="s1p")
        for grp in range(NGRP):
            if grp in hooks:
                hooks[grp]()
            t0 = grp * NWIN
            cnt = min(NWIN, T - t0)
            et = entp.tile([P, NWIN, 4, P], b16, tag="et")
            nc.sync.dma_start(et[:], entT[grp])
            pss = ps_score.tile([P, TPAD], f32, tag="ps_s")
            for k in range(cnt):
                for dk in range(4):
                    nc.tensor.matmul(out=pss[:, 32 * k:32 * k + 32],
                                     lhsT=et[:, k, dk, :],
                                     rhs=tailT[:, dk * R:(dk + 1) * R],
                                     start=(dk == 0), stop=(dk == 3))
            nc.vector.tensor_copy(out=score3[:, t0:t0 + cnt, :],
                                  in_=pss[:, :32 * cnt])
            if grp == NGRP - 1:
                nc.vector.tensor_scalar(out=score3[:, T - 1:T, :],
                                        in0=score3[:, T - 1:T, :],
                                        scalar1=padm_sb[:], scalar2=None,
                                        op0=op.add)
            sl_flat = score3[:, t0:t0 + cnt, :].rearrange("p t r -> p (t r)")
            nc.scalar.activation(out=sl_flat, in_=sl_flat, func=Exp)
            nc.vector.tensor_reduce(
                out=s1p[:, grp, :],
                in_=score3[:, t0:t0 + cnt, :].rearrange("p t r -> p r t"),
                axis=mybir.AxisListType.X, op=op.add)

        # ---- normalizers: c = max(CLIP, sum_sym) / sum_exp per row,
        # applied to score3 only; leftover per-row factors cancel in drcp.
        s1 = small.tile([P, R], f32, tag="s1")
        nc.vector.tensor_reduce(out=s1[:], in_=s1p[:].rearrange("p g r -> p r g"),
                                axis=mybir.AxisListType.X, op=op.add)
        do_y1q(3)
        nc.vector.tensor_reduce(out=y1[:], in_=y1q[:].rearrange("p q r -> p r q"),
                                axis=mybir.AxisListType.X, op=op.add)
        pt_s = ps_tr.tile([P, 512], f32, tag="tr")
        nc.tensor.transpose(out=pt_s[:R, :P], in_=s1[:], identity=ident_sb[:])
        sv = small.tile([R, 1], f32, tag="sv")
        nc.vector.tensor_reduce(out=sv[:], in_=pt_s[:R, :P],
                                axis=mybir.AxisListType.X, op=op.add)
        pt_y = ps_tr.tile([P, 512], f32, tag="tr")
        nc.tensor.transpose(out=pt_y[:R, :P], in_=y1[:], identity=ident_sb[:])
        yv = small.tile([R, 1], f32, tag="yv")
        nc.vector.tensor_reduce(out=yv[:], in_=pt_y[:R, :P],
                                axis=mybir.AxisListType.X, op=op.add)
        nc.vector.tensor_scalar(out=yv[:], in0=yv[:], scalar1=float(CLIP),
                                scalar2=None, op0=op.max)
        sr = small.tile([R, 1], f32, tag="sr")
        nc.vector.reciprocal(out=sr[:], in_=sv[:])
        cv = small.tile([R, 1], b16, tag="cv")
        nc.vector.tensor_tensor(out=cv[:], in0=yv[:], in1=sr[:], op=op.mult)
        cB = ps_trb.tile([P, 512], b16, tag="trb")
        nc.tensor.transpose(out=cB[:, :R], in_=cv[:].to_broadcast([R, P]),
                            identity=identb_sb[:R, :R])
        def bcastc(ps_tile, r0, cc, nr):
            a = ps_tile[:, r0:r0 + nr]
            return bass.AP(a.tensor, a.offset,
                           [list(a.ap[0]), [0, cc], list(a.ap[1])])

        # ---- agg_b = (sym + e*c0)*(sym + e*c1) in 6 big chunks
        agg_b = big.tile([P, T, BLOC], b16, tag="aggb")
        ACH = 66
        NACH = (T + ACH - 1) // ACH
        ft1 = small.tile([P, ACH, BLOC], b16, tag="ft1")
        ft2 = small.tile([P, ACH, BLOC], b16, tag="ft2")
        g1p = small.tile([P, NACH, BLOC], f32, tag="g1p")
        for ci, c0 in enumerate(range(0, T, ACH)):
            cc = min(ACH, T - c0)
            sl = slice(c0, c0 + cc)
            nc.vector.tensor_tensor(out=ft1[:, :cc], in0=score3[:, sl, :BLOC],
                                    in1=bcastc(cB, 0, cc, BLOC), op=op.mult)
            nc.vector.tensor_tensor(out=ft1[:, :cc], in0=ft1[:, :cc],
                                    in1=sym3[:, sl, :BLOC], op=op.add)
            nc.vector.tensor_tensor(out=ft2[:, :cc], in0=score3[:, sl, BLOC:],
                                    in1=bcastc(cB, BLOC, cc, BLOC), op=op.mult)
            nc.vector.tensor_tensor(out=ft2[:, :cc], in0=ft2[:, :cc],
                                    in1=sym3[:, sl, BLOC:], op=op.add)
            nc.vector.tensor_tensor(out=agg_b[:, sl, :], in0=ft1[:, :cc],
                                    in1=ft2[:, :cc], op=op.mult)
            nc.vector.tensor_reduce(
                out=g1p[:, ci, :],
                in_=agg_b[:, sl, :].rearrange("p t r -> p r t"),
                axis=mybir.AxisListType.X, op=op.add)

        # ---- projection loop: pure DMA + matmul
        pso = ps_proj.tile([BLOC, D], f32)
        enp = enpool[0]
        for g in range(NPGRP):
            nt0 = 16 * g
            cnt16 = min(16, T - nt0)
            en = enp.tile([P, 16, D], b16, tag="en")
            nc.sync.dma_start(en[:], entN[g])
            for k in range(cnt16):
                nt = nt0 + k
                nc.tensor.matmul(out=pso[:], lhsT=agg_b[:, nt, :],
                                 rhs=en[:, k, :],
                                 start=(nt == 0), stop=(nt == T - 1))

        # ---- denominator: sum over n of agg_unnorm, then out = pso / denom
        g1 = small.tile([P, BLOC], f32, tag="g1")
        nc.vector.tensor_reduce(out=g1[:],
                                in_=g1p[:].rearrange("p g r -> p r g"),
                                axis=mybir.AxisListType.X, op=op.add)
        pd = ps_tr.tile([P, 512], f32, tag="tr")
        nc.tensor.matmul(out=pd[:BLOC, :1], lhsT=g1[:], rhs=ones128_sb[:],
                         start=True, stop=True)
        dclip = small.tile([BLOC, 1], f32, tag="dclip")
        nc.vector.tensor_scalar(out=dclip[:], in0=pd[:BLOC, :1],
                                scalar1=float(CLIP), scalar2=None, op0=op.max)
        drcp = small.tile([BLOC, 1], f32, tag="drcp")
        nc.vector.reciprocal(out=drcp[:], in_=dclip[:])
        out_sb = small.tile([BLOC, D], f32, tag="outsb")
        nc.vector.tensor_scalar(out=out_sb[:], in0=pso[:], scalar1=drcp[:],
                                scalar2=None, op0=op.mult)
        nc.sync.dma_start(out[:], out_sb[:])

    nc.compile()
    return nc


_PROGRAM = None


def kernel(entity_embedding, head_vector, head_emb, pred_emb,
           edge_val, edge_src, edge_dst):
    global _PROGRAM
    from concourse.bass_utils import run_bass_kernel_spmd

    in_maps = _build_host_inputs(entity_embedding, head_vector,
                                 head_emb, pred_emb,
                                 edge_val, edge_src, edge_dst)
    if _PROGRAM is None:
        _PROGRAM = build_program()
    res = run_bass_kernel_spmd(_PROGRAM, in_maps, list(range(NCORES)))
    out = np.empty((B, D), np.float32)
    for c in range(NCORES):
        out[c * BLOC:(c + 1) * BLOC] = res.results[c]["out"]
    return out


if __name__ == "__main__":
    import reference
    inputs = {k: np.asarray(v) for k, v in reference.setup_inputs().items()}
    got = kernel(**inputs)
    want = np.asarray(reference.reference(**inputs))
    err = np.abs(got - want).max() / np.abs(want).max()
    print("Relative error:", err)
